# revision 68
# baseline (speedup 1.0000x reference)
"""Trainium2 Bass kernel for ModalityAttention (B=4, S=1024, D=2048, H=16, HD=128, RD=64).

Sharding: 8 cores = 4 batches x 2 head-groups (8 heads each).
Each core computes, for its (batch b, head-group g):
  layernorm(x[b]) -> qkv projection (modulation host-folded into the
  weights) for its 8 heads -> rmsnorm + rope -> attention
  -> partial out-projection (transposed layout) with gate folded in.
Host gathers: out[b] = (partial_g0 + partial_g1).T + x[b]

Modulation fold: qkv(xmod) with xmod=(1+s)*xn+b equals
xn @ ((1+s)[d] * W) + b @ W.  The row-scale goes into the host fp8
weight cast (per-core batch); the additive row b@W rides the matmul as
a 17th contraction tile (fp8 ones in xnT x replicated badd/128 rows)
for q/k, and for v folds into the output bias via sum(softmax)=1:
adding c[col] to v shifts o by exactly c, i.e. out by (c @ wo) -- a
host-side vb adjustment.  qkv_b folds into the same terms.

Precision strategy (fp8 e4m3 DoubleRow matmuls where softmax/rms
renormalization absorbs the quantization noise, near-exact elsewhere):
  - q,k projections: fp8 DoubleRow (xn fp8, weights fp8*256).
  - v projection: fp8 DoubleRow with the weight split into hi+lo fp8
    at a shared scale, accumulating in one PSUM group -> ~0.1% weight
    error (the v path feeds the output linearly, it cannot absorb noise).
  - attn*v: fp8 DoubleRow with v split hi+lo on device; exp emits fp8
    with a constant -C_EXP bias (cancels in the softmax ratio) to stay
    under the e4m3 +-240 overflow.
  - softmax denominator: ones-vector DoubleRow matmul on PE whose
    replicated output rows make the broadcast free.
  - scores and out projection: bf16.

Engine balance: gpsimd (Pool) takes SBUF-only elementwise work (k
squares, part of q rope) since it cannot touch PSUM; evacs batch 4
transposes per PSUM tile to amortize the fp8 no-2x penalty on DVE/ACT.
"""
import os, sys

for _p in ("/opt/trn_rl_repo", "/root/.axon_site/_ro/trn_rl_repo", "/root/.axon_site"):
    if os.path.isdir(_p) and _p not in sys.path:
        sys.path.insert(0, _p)

import numpy as np
import ml_dtypes
import concourse.bass as bass
import concourse.bacc as bacc
import concourse.mybir as mybir
import concourse.tile as tile
from concourse import bass_isa
from concourse.masks import make_identity
from concourse.bass_utils import run_bass_kernel_spmd

F32 = mybir.dt.float32
BF16 = mybir.dt.bfloat16
F8 = mybir.dt.float8e4
DR = mybir.MatmulPerfMode.DoubleRow
AF = mybir.ActivationFunctionType
ALU = mybir.AluOpType
WS = 256.0      # host-side qkv weight scale (fp8 e4m3 subnormal floor)
WS2 = 256.0     # host-side out-proj weight scale (descale rides the gate)
C_EXP = 2.0     # exp(score - C_EXP): keeps fp8 attn weights under 240
S, D, HG, HD, RD = 1024, 2048, 8, 128, 64
NT = S // 128        # 8 s-tiles
KT = D // 128        # 16 d-tiles
KTB = KT + 2         # + two ones/badd bias tiles for q,k (a DoubleRow pair)
GCOLS = HG * HD      # 1024 columns per group per projection
EPS = 1e-6
N_CORES = 8


def build_nc(has_norm_w: bool):
    nc = bacc.Bacc("TRN2", target_bir_lowering=False, debug=False,
                   enable_asserts=True, num_devices=N_CORES)

    x = nc.dram_tensor("x", [S, D], BF16, kind="ExternalInput").ap()
    cos = nc.dram_tensor("cos", [S, RD // 2], BF16, kind="ExternalInput").ap()
    sin = nc.dram_tensor("sin", [S, RD // 2], BF16, kind="ExternalInput").ap()
    wq = nc.dram_tensor("wq", [KTB * 128, GCOLS], F8, kind="ExternalInput").ap()
    wk = nc.dram_tensor("wk", [KTB * 128, GCOLS], F8, kind="ExternalInput").ap()
    wvh = nc.dram_tensor("wvh", [KTB * 128, GCOLS], F8, kind="ExternalInput").ap()
    wvl = nc.dram_tensor("wvl", [KTB * 128, GCOLS], F8, kind="ExternalInput").ap()
    # wo pre-laid-out on host as an fp8 hi/lo pair:
    # wo[p, m, hl, kb, c] = hi/lo split of wo[kb*128+p, m*128+c] * WS2
    wo = nc.dram_tensor("wo", [128, KT, 2, HG, 128], F8,
                        kind="ExternalInput").ap()
    gate = nc.dram_tensor("gate", [128, KT], F32, kind="ExternalInput").ap()
    vb = nc.dram_tensor("vb", [128, KT], F32, kind="ExternalInput").ap()
    if has_norm_w:
        wqn = nc.dram_tensor("wqn", [HD], BF16, kind="ExternalInput").ap()
        wkn = nc.dram_tensor("wkn", [HD], BF16, kind="ExternalInput").ap()
    # bf16 output halves the store DMA; the host gather sums the two
    # head-group partials in f32 (bf16 partial rounding ~1e-4 of the answer)
    out_t = nc.dram_tensor("out_t", [D, S], BF16, kind="ExternalOutput").ap()

    with tile.TileContext(nc) as tc:
        # ======== LEFT stack bottom: small persistent constants ====================
        misc_cm = tc.tile_pool(name="misc", bufs=1, side="left")
        misc = misc_cm.__enter__()
        ident_bf = misc.tile([128, 128], BF16)
        eps_t = misc.tile([128, 1], F32)
        eps128_t = misc.tile([128, 1], F32)
        # full-width fp8 ones: the DoubleRow denominator matmul then emits the
        # softmax sum replicated across all 128 partitions (same cost — matmul
        # time only scales with the moving free size), so no broadcast needed
        ones8 = misc.tile([128, 2, 128], F8)
        negc_t = misc.tile([128, 1], F32)
        ones_row = misc.tile([1, 128], BF16)

        def init_misc():
            # emitted after the first x DMAs so the load starts immediately
            make_identity(nc, ident_bf)
            nc.vector.memset(eps_t, EPS)
            nc.vector.memset(eps128_t, HD * EPS)
            nc.vector.memset(ones8, 1.0)
            nc.vector.memset(negc_t, -C_EXP)
            nc.vector.memset(ones_row, 1.0)
        gate_sb = misc.tile([128, KT], F32)
        vb_sb = misc.tile([128, KT], F32)
        rrk_all = misc.tile([128, NT, HG], F32)   # scaled k-rms reciprocals
        if has_norm_w:
            wqn_b = misc.tile([128, HG, HD], BF16)
            wkn_b = misc.tile([128, HG, HD], BF16)
        cos_all = misc.tile([128, NT, RD // 2], BF16)
        sin_all = misc.tile([128, NT, RD // 2], BF16)
        cs_tiles = [(cos_all[:, m, :], sin_all[:, m, :]) for m in range(NT)]

        # ======== RIGHT stack: natural-layout q/k (bf16) and v hi/lo (fp8) =========
        v_cm = tc.tile_pool(name="vpool", bufs=1, side="right")
        v_p = v_cm.__enter__()
        v2_hi = v_p.tile([128, NT, GCOLS], F8)
        v2_lo = v_p.tile([128, NT, GCOLS], F8)
        vtmp = v_p.tile([128, NT, GCOLS], BF16)
        natqk_cm = tc.tile_pool(name="natqk", bufs=1, side="right")
        natqk = natqk_cm.__enter__()
        qnat = natqk.tile([128, NT, GCOLS], BF16)
        knat = natqk.tile([128, NT, GCOLS], BF16)

        # resident qkv weights, triple-buffered: wv_hi/wv_lo both arrive
        # during B-k (tile 3 reuses tile 0's space, freed after B-q)
        wres_cm = tc.tile_pool(name="wres", bufs=3, side="right")
        wres_p = wres_cm.__enter__()
        wres_t = [wres_p.tile([128, KTB, GCOLS], F8, tag="wres", name=f"wres{pi}")
                  for pi in range(4)]
        wq_r = wq.rearrange("(k p) c -> p k c", p=128)
        wk_r = wk.rearrange("(k p) c -> p k c", p=128)
        wvh_r = wvh.rearrange("(k p) c -> p k c", p=128)
        wvl_r = wvl.rearrange("(k p) c -> p k c", p=128)
        w_drams = (wq_r, wk_r, wvh_r, wvl_r)

        def load_wres(pi, chunks=range(4)):
            for c4 in chunks:
                lo = c4 * 4
                hi = KTB if c4 == 3 else lo + 4   # last chunk carries tile 16
                nc.sync.dma_start(out=wres_t[pi][:, lo:hi, :],
                                  in_=w_drams[pi][:, lo:hi, :])

        # ======== phase A pools ====================================================
        xnT_cm = tc.tile_pool(name="xnT", bufs=1, side="left")
        xnT_p = xnT_cm.__enter__()
        xnT = xnT_p.tile([128, KTB, S], F8)  # [d_in_tile, d_tile, s]
        # bias-tile pair: tile16 = ones (x badd row), tile17 = 16*nmr per
        # column (x colsum(W)/16 row) — the mean-subtraction term
        # (-mean*rstd*colsum) is rank-1 and rides the same DoubleRow pair
        nc.gpsimd.memset(xnT[:, KT, :], 1.0)

        a_cm = tc.tile_pool(name="phA", bufs=3, side="left")
        a_p = a_cm.__enter__()
        a_small_cm = tc.tile_pool(name="phA_small", bufs=6, side="left")
        a_small = a_small_cm.__enter__()
        pst_cm = tc.tile_pool(name="ps_tr", bufs=4, space="PSUM")
        pst = pst_cm.__enter__()
        # pt: 4 bufs x 1 bank; ptn: 1 buf -> 5 banks + ps_qkv's 3 = 8

        # phase C pools opened early so later emission can overlap B on DVE
        c_cm = tc.tile_pool(name="phC", bufs=2, side="left")
        c_p = c_cm.__enter__()
        c_small_cm = tc.tile_pool(name="phC_small", bufs=2, side="left")
        c_small = c_small_cm.__enter__()

        psb_cm = tc.tile_pool(name="ps_qkv", bufs=3, space="PSUM")
        psb = psb_cm.__enter__()

        # ---- emit helpers ----
        def emit_A_dma(i):
            xt = a_p.tile([128, D], BF16, tag="xt", name=f"xt{i}")
            nc.sync.dma_start(out=xt[:, 0:D // 2],
                              in_=x[i * 128:(i + 1) * 128, 0:D // 2])
            nc.sync.dma_start(out=xt[:, D // 2:D],
                              in_=x[i * 128:(i + 1) * 128, D // 2:D])
            return xt

        def emit_A_stats(i, xt=None):
            """x load + LN statistics; emitted 2 s-tiles ahead of the body so
            the serial stats chain never gates the PE transposes."""
            if xt is None:
                xt = emit_A_dma(i)
            stats = a_small.tile([128, 4, 6], F32, tag="stats",
                                 name=f"stats{i}")
            xv = xt.rearrange("p (c f) -> p c f", c=4)
            for c in range(4):
                nc.vector.bn_stats(out=stats[:, c, :], in_=xv[:, c, :])
            mv = a_small.tile([128, 2], F32, tag="mv", name=f"mv{i}")
            nc.vector.bn_aggr(out=mv, in_=stats)
            rstd = a_small.tile([128, 1], F32, tag="rstd", name=f"rstd{i}")
            nc.scalar.activation(out=rstd, in_=mv[:, 1:2], func=AF.Sqrt,
                                 bias=eps_t, scale=1.0)
            nc.vector.reciprocal(out=rstd, in_=rstd)
            nmr = a_small.tile([128, 1], F32, tag="nmr", name=f"nmr{i}")
            nc.vector.tensor_scalar(out=nmr, in0=mv[:, 0:1], scalar1=rstd,
                                    scalar2=-1.0, op0=ALU.mult, op1=ALU.mult)
            # LN-apply rides the transpose matmul: rhs = diag(rstd); the
            # -mean*rstd term lands in xnT's 17th column tile (16*nmr in fp8,
            # contracted with host colsum(W)/16 rows by the bias DR pair)
            diag_t = a_small.tile([128, 128], BF16, tag="diag",
                                  name=f"diag{i}")
            nc.vector.tensor_scalar_mul(out=diag_t, in0=ident_bf,
                                        scalar1=rstd)
            nmr_bf = a_small.tile([128, 1], BF16, tag="nmr_bf",
                                  name=f"nmr_bf{i}")
            nc.vector.tensor_scalar_mul(out=nmr_bf, in0=nmr, scalar1=16.0)
            ptn = pst.tile([1, 128], BF16, tag="ptn", name=f"ptn{i}", bufs=1)
            nc.tensor.transpose(ptn, nmr_bf, ident_bf)
            nmr_row = a_small.tile([1, 128], BF16, tag="nmr_row",
                                   name=f"nmr_row{i}")
            nc.vector.tensor_copy(out=nmr_row, in_=ptn)
            # partition-broadcast via a rank-1 PE matmul (nothing else can
            # replicate a row across partitions) into a pt-ring slot
            pbc = pst.tile([128, 4, 128], F32, tag="pt", name=f"pbc{i}")
            nc.tensor.matmul(pbc[:, 0, :], ones_row, nmr_row,
                             start=True, stop=True)
            nc.vector.tensor_copy(out=xnT[:, KT + 1, i * 128:(i + 1) * 128],
                                  in_=pbc[:, 0, :])
            return xt, diag_t

        def emit_A_body(i, xt, diag_t):
            # 4 normalize-transposes per psum tile -> one wide plain-copy
            # evac (modulation lives in the host-folded weights)
            for k4 in range(KT // 4):
                pt = pst.tile([128, 4, 128], F32, tag="pt")
                for j in range(4):
                    k = k4 * 4 + j
                    # j=0's start zeroes the whole 2KB PSUM zero-region; a
                    # start on later j would re-mark earlier slices pending-
                    # zero for the region-spanning group
                    nc.tensor.matmul(pt[:, j, :], xt[:, k * 128:(k + 1) * 128],
                                     diag_t, start=(j == 0), stop=(j == 3),
                                     skip_group_check=True)
                dst = xnT[:, k4 * 4:k4 * 4 + 4, i * 128:(i + 1) * 128]
                nc.scalar.copy(out=dst, in_=pt)

        def emit_Bm_qk(pi, nat, m, evac_engines):
            # last DoubleRow pair is the two ones-tiles x replicated badd/256
            # rows, adding the modulation/bias term
            wt = wres_t[pi]
            for n in range(2):
                ps = psb.tile([128, 512], F32, tag="ps")
                for k in range(0, KTB, 2):
                    nc.tensor.matmul(ps, xnT[:, k:k + 2, m * 128:(m + 1) * 128],
                                     wt[:, k:k + 2, n * 512:(n + 1) * 512],
                                     start=(k == 0), stop=(k == KTB - 2),
                                     perf_mode=DR)
                dst = nat[:, m, n * 512:(n + 1) * 512]
                if evac_engines[n] == "act":
                    nc.scalar.activation(out=dst, in_=ps, func=AF.Identity,
                                         scale=1.0 / WS)
                else:
                    nc.vector.tensor_scalar_mul(out=dst, in0=ps,
                                                scalar1=1.0 / WS)

        def emit_Bm_v(m):
            # v = xn @ (wv_hi + wv_lo), both chains in one PSUM group; evac
            # to bf16 vtmp (ACT), then fp8 hi (ACT); the lo split is deferred
            # to the D/E boundary where DVE has slack.
            for n in range(2):
                ps = psb.tile([128, 512], F32, tag="ps")
                for k in range(0, KT, 2):
                    nc.tensor.matmul(ps, xnT[:, k:k + 2, m * 128:(m + 1) * 128],
                                     wres_t[2][:, k:k + 2, n * 512:(n + 1) * 512],
                                     start=(k == 0), stop=False, perf_mode=DR)
                    nc.tensor.matmul(ps, xnT[:, k:k + 2, m * 128:(m + 1) * 128],
                                     wres_t[3][:, k:k + 2, n * 512:(n + 1) * 512],
                                     start=False, stop=False, perf_mode=DR)
                # bias pair for the -mean*rstd*colsum(wv) term (badd_v itself
                # folds into vb via sum(softmax)=1)
                nc.tensor.matmul(ps, xnT[:, KT:KT + 2, m * 128:(m + 1) * 128],
                                 wres_t[2][:, KT:KT + 2, n * 512:(n + 1) * 512],
                                 start=False, stop=True, perf_mode=DR)
                sl = slice(n * 512, (n + 1) * 512)
                nc.scalar.activation(out=vtmp[:, m, sl], in_=ps,
                                     func=AF.Identity, scale=1.0 / WS)
            nc.scalar.copy(out=v2_hi[:, m, :], in_=vtmp[:, m, :])

        def emit_Cq(m):
            qm = qnat[:, m, :]
            qmh = qm.rearrange("p (h c) -> p h c", h=HG)
            (ct, st) = cs_tiles[m]
            cb = ct.unsqueeze(1).broadcast_to([128, HG, RD // 2])
            sb_ = st.unsqueeze(1).broadcast_to([128, HG, RD // 2])
            sq = c_p.tile([128, GCOLS], BF16, tag="sqk")
            nc.gpsimd.tensor_mul(out=sq, in0=qm, in1=qm)
            ssq = c_small.tile([128, HG], BF16, tag="ssq")
            with nc.allow_low_precision(reason="bf16 rms stats; ~0.3% scale"):
                nc.vector.reduce_sum(out=ssq,
                                     in_=sq.rearrange("p (h d) -> p h d", h=HG),
                                     axis=mybir.AxisListType.X)
            rrq = c_small.tile([128, HG], BF16, tag="rrq")
            nc.scalar.activation(out=rrq, in_=ssq, func=AF.Sqrt,
                                 bias=eps_t, scale=1.0 / HD)
            with nc.allow_low_precision(reason="bf16 1/rms scale; 0.4% rel"):
                nc.vector.reciprocal(out=rrq, in_=rrq)
            if has_norm_w:
                nc.vector.tensor_mul(out=qmh, in0=qmh, in1=wqn_b)
            _emit_rope(qmh, cb, sb_, pool_half=True)
            rrq_b = rrq.unsqueeze(2).broadcast_to([128, HG, HD])
            nc.vector.tensor_mul(out=qmh, in0=qmh, in1=rrq_b)

        def emit_Ck(m):
            km = knat[:, m, :]
            kmh = km.rearrange("p (h c) -> p h c", h=HG)
            (ct, st) = cs_tiles[m]
            cb = ct.unsqueeze(1).broadcast_to([128, HG, RD // 2])
            sb_ = st.unsqueeze(1).broadcast_to([128, HG, RD // 2])
            sk_ = c_p.tile([128, GCOLS], BF16, tag="sqk")
            # square on the idle gpsimd engine: DVE is the pacer here
            nc.gpsimd.tensor_mul(out=sk_, in0=km, in1=km)
            ssk = c_small.tile([128, HG], BF16, tag="ssk")
            with nc.allow_low_precision(reason="bf16 rms stats; ~0.3% scale"):
                nc.vector.reduce_sum(out=ssk,
                                     in_=sk_.rearrange("p (h d) -> p h d", h=HG),
                                     axis=mybir.AxisListType.X)
            nc.scalar.activation(out=rrk_all[:, m, :], in_=ssk, func=AF.Sqrt,
                                 bias=eps128_t, scale=1.0)
            nc.vector.reciprocal(out=rrk_all[:, m, :], in_=rrk_all[:, m, :])
            if has_norm_w:
                nc.vector.tensor_mul(out=kmh, in0=kmh, in1=wkn_b)
            _emit_rope(kmh, cb, sb_, pool_half=False)

        def _emit_rope(mh, cb, sb_, pool_half):
            # host-de-interleaved: cols [0:32]=x0, [32:64]=x1 per head.
            # pool_half routes the y1 branch to gpsimd (free in the q region).
            x0 = mh[:, :, 0:RD // 2]
            x1 = mh[:, :, RD // 2:RD]
            t0 = c_small.tile([128, HG, RD // 2], BF16, tag="t0")
            t1 = c_small.tile([128, HG, RD // 2], BF16, tag="t1")
            t2 = c_small.tile([128, HG, RD // 2], BF16, tag="t2")
            t3 = c_small.tile([128, HG, RD // 2], BF16, tag="t3")
            e = nc.gpsimd if pool_half else nc.vector
            nc.vector.tensor_mul(out=t0, in0=x0, in1=cb)
            nc.vector.tensor_mul(out=t1, in0=x1, in1=sb_)
            e.tensor_mul(out=t2, in0=x0, in1=sb_)
            e.tensor_mul(out=t3, in0=x1, in1=cb)
            nc.vector.tensor_sub(out=x0, in0=t0, in1=t1)
            e.tensor_add(out=x1, in0=t2, in1=t3)

        # ---- fused A + B-q emission ----
        xt01 = {i: emit_A_dma(i) for i in (0, 1)}
        init_misc()
        pend = {i: emit_A_stats(i, xt01.pop(i)) for i in (0, 1)}
        for i in range(NT):
            if i + 2 < NT:
                pend[i + 2] = emit_A_stats(i + 2)
            emit_A_body(i, *pend.pop(i))
            # all wq chunks MUST be emitted before the first emit_Bm below
            # (deps only order reads after already-emitted writes)
            if i == 0:
                load_wres(0, range(0, 2))
            elif i == 1:
                load_wres(0, range(2, 4))
            elif i == 3:
                load_wres(1)          # wk, consumed after A+B-q
            if i >= 1:
                emit_Bm_qk(0, qnat, i - 1, ("act", "act"))
        emit_Bm_qk(0, qnat, NT - 1, ("act", "act"))

        # deferred misc loads (consumed in phases C/E/F) — single batched DMAs
        # so their dispatch overhead doesn't contend with the x/w streams
        nc.sync.dma_start(out=gate_sb, in_=gate)
        nc.sync.dma_start(out=vb_sb, in_=vb)
        if has_norm_w:
            def _bcast_from_dram(ap, parts, reps=None):
                newap = [[0, parts]]
                if reps is not None:
                    newap.append([0, reps])
                newap += list(ap.ap)
                return bass.AP(tensor=ap.tensor, offset=ap.offset, ap=newap)
            nc.sync.dma_start(out=wqn_b, in_=_bcast_from_dram(wqn, 128, reps=HG))
            nc.sync.dma_start(out=wkn_b, in_=_bcast_from_dram(wkn, 128, reps=HG))
        nc.sync.dma_start(out=cos_all, in_=cos.rearrange("(m p) f -> p m f", p=128))
        nc.sync.dma_start(out=sin_all, in_=sin.rearrange("(m p) f -> p m f", p=128))

        # ---- B-k (PE) interleaved with C-q (DVE) per m-tile ----
        for m in range(NT):
            emit_Bm_qk(1, knat, m, ("act", "act"))
            emit_Cq(m)
            if m == 0:
                load_wres(2)          # wv_hi
                load_wres(3)          # wv_lo
        # ---- B-v (PE) interleaved with C-k (DVE) ----
        for m in range(NT):
            emit_Bm_v(m)
            emit_Ck(m)

        psb_cm.__exit__(None, None, None)
        pst_cm.__exit__(None, None, None)
        c_small_cm.__exit__(None, None, None)
        c_cm.__exit__(None, None, None)
        a_small_cm.__exit__(None, None, None)
        a_cm.__exit__(None, None, None)
        xnT_cm.__exit__(None, None, None)
        wres_cm.__exit__(None, None, None)

        # ======== phases D/E/F share the left stack ================================
        oT_cm = tc.tile_pool(name="oT", bufs=1, side="left")
        oT_p = oT_cm.__enter__()
        oT_hi = oT_p.tile([128, HG, S], F8)
        oT_lo = oT_p.tile([128, HG, S], F8)

        # wo prefetch (resident; consumed in phase F)
        wo_cm = tc.tile_pool(name="wo_res", bufs=1, side="left")
        wo_p = wo_cm.__enter__()
        wo_hi_sb = wo_p.tile([128, KT, HG, 128], F8)
        wo_lo_sb = wo_p.tile([128, KT, HG, 128], F8)

        # ---- phase D: transpose q, k -> [hd, s] per head (batched evacs)
        # q evacs on ACT (DVE is finishing C-k); k evacs on DVE; the v lo
        # split (SBUF-only, deferred from B-v) fills DVE alongside.
        qkT_cm = tc.tile_pool(name="qkT", bufs=1, side="left")
        qkT_p = qkT_cm.__enter__()
        qT = qkT_p.tile([128, HG, S], BF16)
        kT = qkT_p.tile([128, HG, S], BF16)
        pst2_cm = tc.tile_pool(name="ps_tr2", bufs=4, space="PSUM")
        pst2 = pst2_cm.__enter__()

        def emit_D(nat, dstT, h, evac):
            pt8 = pst2.tile([128, S], BF16, tag="pt8")
            for m in range(NT):
                nc.tensor.transpose(pt8[:, m * 128:(m + 1) * 128],
                                    nat[:, m, h * 128:(h + 1) * 128],
                                    ident_bf)
            dst = dstT[:, h, :]
            if evac == "act":
                nc.scalar.copy(out=dst, in_=pt8)
            else:
                nc.vector.tensor_copy(out=dst, in_=pt8)

        for h in range(HG):
            emit_D(qnat, qT, h, "act")
            if h % 2 == 0:
                m = h // 2 * 2
                with nc.allow_low_precision(reason="fp8 v lo split"):
                    nc.vector.tensor_sub(out=v2_lo[:, m, :],
                                         in0=vtmp[:, m, :], in1=v2_hi[:, m, :])
                    nc.vector.tensor_sub(out=v2_lo[:, m + 1, :],
                                         in0=vtmp[:, m + 1, :],
                                         in1=v2_hi[:, m + 1, :])
        for h in range(HG):
            emit_D(knat, kT, h, "dve")
        pst2_cm.__exit__(None, None, None)
        # (natqk stays open through E/F: closing it between D and E would
        # add a pool barrier; SBUF has room)

        # wo loads (consumed in F; DMA-engine time hidden under phase E)
        for m in range(KT):
            nc.sync.dma_start(out=wo_hi_sb[:, m, :, :], in_=wo[:, m, 0, :, :])
            nc.sync.dma_start(out=wo_lo_sb[:, m, :, :], in_=wo[:, m, 1, :, :])

        # ---- phase E: attention per head (fp8 DoubleRow attn*v) ----
        at2_cm = tc.tile_pool(name="at2", bufs=2, side="left")
        at2_p = at2_cm.__enter__()
        rs_cm = tc.tile_pool(name="rsb", bufs=2, side="left")
        rs_p = rs_cm.__enter__()
        pssc_cm = tc.tile_pool(name="ps_sc", bufs=2, space="PSUM")
        pssc = pssc_cm.__enter__()
        pso_cm = tc.tile_pool(name="ps_o", bufs=1, space="PSUM")
        pso = pso_cm.__enter__()
        psdn_cm = tc.tile_pool(name="ps_dn", bufs=2, space="PSUM")
        psdn = psdn_cm.__enter__()

        def emit_scores(h, m):
            sc = pssc.tile([128, S], F32, tag="sc")
            lhs_k = kT[:, h, m * 128:(m + 1) * 128]
            nc.tensor.matmul(sc[:, 0:512], lhs_k, qT[:, h, 0:512],
                             start=True, stop=True)
            nc.tensor.matmul(sc[:, 512:1024], lhs_k, qT[:, h, 512:1024],
                             start=True, stop=True)
            return sc

        for h in range(HG):
            o_ps = pso.tile([128, S], F32, tag="o_ps", name=f"o_ps{h}")
            dns = [psdn.tile([128, 512], F32, tag="dn", name=f"dn{h}_{g}")
                   for g in range(2)]
            sc_next = emit_scores(h, 0)
            at = None
            for m in range(NT):
                sc = sc_next
                sc_next = emit_scores(h, m + 1) if m + 1 < NT else None
                if m % 2 == 0:
                    at = at2_p.tile([128, 2, S], F8, tag="at", name="at")
                # fp8 attn weights: exp(score*rrk - C) stays under e4m3 240;
                # the e^-C factor cancels between numerator and denominator
                nc.scalar.activation(out=at[:, m % 2, :], in_=sc, func=AF.Exp,
                                     scale=rrk_all[:, m, h:h + 1], bias=negc_t)
                if m % 2 == 1:
                    mp = m // 2
                    first, last = (mp == 0), (mp == NT // 2 - 1)
                    for g in range(2):
                        # denominator on PE: ones^T @ at, replicated rows
                        nc.tensor.matmul(dns[g], ones8,
                                         at[:, :, g * 512:(g + 1) * 512],
                                         start=first, stop=last, perf_mode=DR)
                    for vi, vsrc in enumerate((v2_hi, v2_lo)):
                        v_mh = vsrc[:, 2 * mp:2 * mp + 2, h * 128:(h + 1) * 128]
                        nc.tensor.matmul(o_ps[:, 0:512], v_mh, at[:, :, 0:512],
                                         start=(first and vi == 0),
                                         stop=(last and vi == 1), perf_mode=DR)
                        nc.tensor.matmul(o_ps[:, 512:1024], v_mh,
                                         at[:, :, 512:1024],
                                         start=(first and vi == 0),
                                         stop=(last and vi == 1), perf_mode=DR)
            for g in range(2):
                sl = slice(g * 512, (g + 1) * 512)
                sums = rs_p.tile([128, 512], F32, tag="sums", name=f"sums{h}{g}")
                nc.vector.reciprocal(out=sums, in_=dns[g])
                obf = rs_p.tile([128, 512], BF16, tag="obf", name=f"obf{h}{g}")
                nc.vector.tensor_mul(out=obf, in0=o_ps[:, sl], in1=sums)
                nc.vector.tensor_copy(out=oT_hi[:, h, sl], in_=obf)
                with nc.allow_low_precision(reason="fp8 o lo split"):
                    nc.vector.tensor_sub(out=oT_lo[:, h, sl], in0=obf,
                                         in1=oT_hi[:, h, sl])

        # ---- phase F: out projection as three fp8 DoubleRow chains
        # (hi*hi + hi*lo + lo*hi; the dropped lo*lo term is ~0.1%^2), emitted
        # inside E's psum pools so no pool barrier splits E/F
        F_CHAINS = 3
        F_KP = HG // 2

        def emit_F_mm(po, m, ci, kp):
            wsrc, osrc = ((wo_hi_sb, oT_hi), (wo_hi_sb, oT_lo),
                          (wo_lo_sb, oT_hi))[ci]
            first = (ci == 0 and kp == 0)
            last = (ci == F_CHAINS - 1 and kp == F_KP - 1)
            lhs = wsrc[:, m, 2 * kp:2 * kp + 2, :]
            nc.tensor.matmul(po[:, 0:512], lhs, osrc[:, 2 * kp:2 * kp + 2, 0:512],
                             start=first, stop=last, perf_mode=DR)
            nc.tensor.matmul(po[:, 512:1024], lhs,
                             osrc[:, 2 * kp:2 * kp + 2, 512:1024],
                             start=first, stop=last, perf_mode=DR)

        def emit_F_evac(po, m):
            ot_t = rs_p.tile([128, S], BF16, tag="ot_t", name=f"ot_t{m}")
            if m == KT - 1:
                # the kernel's tail: halve the evac+store chain so the first
                # half's DMA overlaps the second half's evacuation
                for g in (0, 1):
                    sl = slice(g * 512, (g + 1) * 512)
                    nc.vector.tensor_scalar(out=ot_t[:, sl], in0=po[:, sl],
                                            scalar1=gate_sb[:, m:m + 1],
                                            scalar2=vb_sb[:, m:m + 1],
                                            op0=ALU.mult, op1=ALU.add)
                    nc.sync.dma_start(out=out_t[m * 128:(m + 1) * 128, sl],
                                      in_=ot_t[:, sl])
                return
            if m % 2 == 0:
                nc.scalar.activation(out=ot_t, in_=po, func=AF.Identity,
                                     bias=vb_sb[:, m:m + 1],
                                     scale=gate_sb[:, m:m + 1])
            else:
                nc.vector.tensor_scalar(out=ot_t, in0=po,
                                        scalar1=gate_sb[:, m:m + 1],
                                        scalar2=vb_sb[:, m:m + 1],
                                        op0=ALU.mult, op1=ALU.add)
            nc.sync.dma_start(out=out_t[m * 128:(m + 1) * 128, :], in_=ot_t)

        def alloc_po(m):
            # alternate between the two dead attention psum pools for slack
            return (pssc.tile([128, S], F32, tag="sc", name=f"po{m}")
                    if m % 2 == 0
                    else pso.tile([128, S], F32, tag="o_ps", name=f"po{m}"))

        # first two output tiles interleave their chain loops so F has dense
        # matmuls queued before it needs the last head's normalization
        steps = [(ci, kp) for ci in range(F_CHAINS) for kp in range(F_KP)]
        po01 = {m: alloc_po(m) for m in (0, 1)}
        for m in (0, 1):
            for (ci, kp) in steps[:-2]:
                emit_F_mm(po01[m], m, ci, kp)
        for m in (0, 1):
            for (ci, kp) in steps[-2:]:
                emit_F_mm(po01[m], m, ci, kp)
            emit_F_evac(po01[m], m)
        for m in range(2, KT):
            po = alloc_po(m)
            for (ci, kp) in steps:
                emit_F_mm(po, m, ci, kp)
            emit_F_evac(po, m)

        psdn_cm.__exit__(None, None, None)
        pso_cm.__exit__(None, None, None)
        pssc_cm.__exit__(None, None, None)
        rs_cm.__exit__(None, None, None)
        at2_cm.__exit__(None, None, None)
        qkT_cm.__exit__(None, None, None)
        natqk_cm.__exit__(None, None, None)
        v_cm.__exit__(None, None, None)
        wo_cm.__exit__(None, None, None)
        oT_cm.__exit__(None, None, None)
        misc_cm.__exit__(None, None, None)

    nc.compile()
    return nc


_NC_CACHE = {}


def _get_nc(has_norm_w):
    if has_norm_w not in _NC_CACHE:
        _NC_CACHE[has_norm_w] = build_nc(has_norm_w)
    return _NC_CACHE[has_norm_w]


# per-head column permutation de-interleaving rope pairs:
# [0,2,..,62, 1,3,..,63, 64..127] within each head's 128 columns
def _qk_perm():
    base = np.concatenate([np.arange(0, RD, 2), np.arange(1, RD, 2),
                           np.arange(RD, HD)])
    return np.concatenate([h * HD + base for h in range(HG)])


_PERM = _qk_perm()
F8NP = ml_dtypes.float8_e4m3


def prep_in_maps(x, mod, cos, sin, qkv_w, qkv_b, mod_w, mod_b, out_w, out_b,
                 norm_q_w, norm_k_w):
    """Host-side sharding. Returns (in_maps, flags, x_np)."""
    x = np.asarray(x, dtype=np.float32)
    m3 = np.asarray(mod, np.float32) @ np.asarray(mod_w, np.float32) \
        + np.asarray(mod_b, np.float32)
    bias, scale, gatef = np.split(m3, 3, axis=-1)          # [B, D] each
    scale1p = (1.0 + scale).astype(np.float32)

    qkv_b = np.asarray(qkv_b, np.float32)
    has_norm_w = not (np.allclose(norm_q_w, 1.0) and np.allclose(norm_k_w, 1.0))

    cosb = np.ascontiguousarray(np.asarray(cos, np.float32).astype(ml_dtypes.bfloat16))
    sinb = np.ascontiguousarray(np.asarray(sin, np.float32).astype(ml_dtypes.bfloat16))
    qkv_w = np.asarray(qkv_w, np.float32)
    out_w = np.asarray(out_w, np.float32)
    out_b = np.asarray(out_b, np.float32)

    # per-head rope-pair de-interleave permutation (shared by q and k; inner
    # products and rms are invariant; cos/sin indices line up with x0/x1 halves)
    perm = _PERM
    hd_perm = np.concatenate([np.arange(0, RD, 2), np.arange(1, RD, 2),
                              np.arange(RD, HD)])

    def with_bias_tile(w_scaled_f8, badd, colsum_scaled):
        # tile16 rows (x ones): badd*WS/128; tile17 rows (x 16*nmr):
        # colsum(W_fp8)*WS / (128*16) — the rank-1 mean-subtraction term
        brow = np.broadcast_to((badd * (WS / 128.0))[None, :],
                               (128, badd.shape[0]))
        crow = np.broadcast_to((colsum_scaled / 2048.0)[None, :],
                               (128, badd.shape[0]))
        return np.ascontiguousarray(np.concatenate(
            [w_scaled_f8, brow.astype(F8NP), crow.astype(F8NP)], axis=0))

    in_maps = []
    for c in range(N_CORES):
        b, g = divmod(c, 2)
        lo = g * GCOLS
        s1p = scale1p[b][:, None]                          # [D, 1]
        wq_c = (qkv_w[:, lo:lo + GCOLS] * s1p)[:, perm]
        wk_c = (qkv_w[:, 2048 + lo:2048 + lo + GCOLS] * s1p)[:, perm]
        wv_c = qkv_w[:, 4096 + lo:4096 + lo + GCOLS] * s1p
        # modulation/qkv bias rows: b @ W (+ qkv bias), per projection
        badd_q = bias[b] @ qkv_w[:, lo:lo + GCOLS] + qkv_b[lo:lo + GCOLS]
        badd_k = bias[b] @ qkv_w[:, 2048 + lo:2048 + lo + GCOLS] \
            + qkv_b[2048 + lo:2048 + lo + GCOLS]
        badd_v = bias[b] @ qkv_w[:, 4096 + lo:4096 + lo + GCOLS] \
            + qkv_b[4096 + lo:4096 + lo + GCOLS]
        wo_c = out_w[lo:lo + GCOLS, :]                      # [GCOLS, D]
        # wo_l[p, m, kb, c] = wo_c[kb*128+p, m*128+c], fp8 hi/lo pair
        wo_l = np.transpose(
            wo_c.reshape(HG, 128, KT, 128), (1, 2, 0, 3)) * WS2
        wo_hi = wo_l.astype(F8NP)
        wo_lo = (wo_l - wo_hi.astype(np.float32)).astype(F8NP)
        wo_pair = np.stack([wo_hi, wo_lo], axis=2)   # [128, KT, 2, HG, 128]
        # v weights as an fp8 hi+lo pair at a shared scale: lo lands in the
        # e4m3 subnormal range, leaving ~0.1% net weight error
        wvh_c = (wv_c * WS).astype(F8NP)
        wvl_c = (wv_c * WS - wvh_c.astype(np.float32)).astype(F8NP)
        zrow = np.zeros((256, GCOLS), np.float32)
        wq_f8 = (wq_c * WS).astype(F8NP)
        wk_f8 = (wk_c * WS).astype(F8NP)
        # v bias folds into the output bias: sum(softmax)=1 makes o shift by
        # badd_v exactly, i.e. the final out shifts by (badd_v @ wo) * gate
        vb_full = badd_v @ wo_c                             # [D]
        if g == 0:
            vb_full = vb_full + out_b
        vbf = (vb_full * gatef[b]).astype(np.float32)
        im = {
            "x": np.ascontiguousarray(x[b].astype(ml_dtypes.bfloat16)),
            "cos": cosb, "sin": sinb,
            "wq": with_bias_tile(wq_f8, badd_q[perm],
                                 wq_f8.astype(np.float32).sum(0)),
            "wk": with_bias_tile(wk_f8, badd_k[perm],
                                 wk_f8.astype(np.float32).sum(0)),
            "wvh": with_bias_tile(wvh_c, np.zeros(GCOLS, np.float32),
                                  wvh_c.astype(np.float32).sum(0)
                                  + wvl_c.astype(np.float32).sum(0)),
            "wvl": np.ascontiguousarray(
                np.concatenate([wvl_c, zrow.astype(F8NP)], axis=0)),
            "wo": np.ascontiguousarray(wo_pair),
            "gate": np.ascontiguousarray(
                (gatef[b] / WS2).reshape(KT, 128).T),
            "vb": np.ascontiguousarray(vbf.reshape(KT, 128).T),
        }
        if has_norm_w:
            im["wqn"] = np.ascontiguousarray(
                np.asarray(norm_q_w, np.float32)[hd_perm].astype(ml_dtypes.bfloat16))
            im["wkn"] = np.ascontiguousarray(
                np.asarray(norm_k_w, np.float32)[hd_perm].astype(ml_dtypes.bfloat16))
        in_maps.append(im)
    return in_maps, has_norm_w, x


def gather(results, x):
    B = x.shape[0]
    outs = []
    for b in range(B):
        p = results[2 * b]["out_t"].astype(np.float32) \
            + results[2 * b + 1]["out_t"].astype(np.float32)        # [D, S]
        outs.append(p.T + x[b])
    return np.stack(outs).astype(np.float32)


def kernel(**inputs) -> np.ndarray:
    in_maps, has_norm_w, x = prep_in_maps(**inputs)
    nc = _get_nc(has_norm_w)
    res = run_bass_kernel_spmd(nc, in_maps, core_ids=list(range(N_CORES)))
    return gather(res.results, x)


if __name__ == "__main__":
    import time
    t0 = time.time()
    nc = build_nc(False)
    print("build+compile ok in", time.time() - t0, "s")


# revision 72
# speedup vs baseline: 1.1404x; 1.1404x over previous
"""Trainium2 Bass kernel for ModalityAttention (B=4, S=1024, D=2048, H=16, HD=128, RD=64).

Sharding: 8 cores = 4 batches x 2 head-groups (8 heads each).
Each core computes, for its (batch b, head-group g):
  layernorm(x[b]) -> qkv projection (modulation host-folded into the
  weights) for its 8 heads -> rmsnorm + rope -> attention
  -> partial out-projection (transposed layout) with gate folded in.
Host gathers: out[b] = (partial_g0 + partial_g1).T + x[b]

Modulation fold: qkv(xmod) with xmod=(1+s)*xn+b equals
xn @ ((1+s)[d] * W) + b @ W.  The row-scale goes into the host fp8
weight cast (per-core batch); the additive row b@W rides the matmul as
a 17th contraction tile (fp8 ones in xnT x replicated badd/128 rows)
for q/k, and for v folds into the output bias via sum(softmax)=1:
adding c[col] to v shifts o by exactly c, i.e. out by (c @ wo) -- a
host-side vb adjustment.  qkv_b folds into the same terms.

Precision strategy (fp8 e4m3 DoubleRow matmuls where softmax/rms
renormalization absorbs the quantization noise, near-exact elsewhere):
  - q,k projections: fp8 DoubleRow (xn fp8, weights fp8*256).
  - v projection: fp8 DoubleRow with the weight split into hi+lo fp8
    at a shared scale, accumulating in one PSUM group -> ~0.1% weight
    error (the v path feeds the output linearly, it cannot absorb noise).
  - attn*v: fp8 DoubleRow with v split hi+lo on device; exp emits fp8
    with a constant -C_EXP bias (cancels in the softmax ratio) to stay
    under the e4m3 +-240 overflow.
  - softmax denominator: ones-vector DoubleRow matmul on PE whose
    replicated output rows make the broadcast free.
  - scores and out projection: bf16.

Engine balance: gpsimd (Pool) takes SBUF-only elementwise work (k
squares, part of q rope) since it cannot touch PSUM; evacs batch 4
transposes per PSUM tile to amortize the fp8 no-2x penalty on DVE/ACT.
"""
import os, sys

for _p in ("/opt/trn_rl_repo", "/root/.axon_site/_ro/trn_rl_repo", "/root/.axon_site"):
    if os.path.isdir(_p) and _p not in sys.path:
        sys.path.insert(0, _p)

import numpy as np
import ml_dtypes
import concourse.bass as bass
import concourse.bacc as bacc
import concourse.mybir as mybir
import concourse.tile as tile
from concourse import bass_isa
from concourse.masks import make_identity
from concourse.bass_utils import run_bass_kernel_spmd

F32 = mybir.dt.float32
BF16 = mybir.dt.bfloat16
F8 = mybir.dt.float8e4
DR = mybir.MatmulPerfMode.DoubleRow
AF = mybir.ActivationFunctionType
ALU = mybir.AluOpType
WS = 256.0      # host-side qkv weight scale (fp8 e4m3 subnormal floor)
WS2 = 256.0     # host-side out-proj weight scale (descale rides the gate)
C_EXP = 2.0     # exp(score - C_EXP): keeps fp8 attn weights under 240
S, D, HG, HD, RD = 1024, 2048, 8, 128, 64
NT = S // 128        # 8 s-tiles
KT = D // 128        # 16 d-tiles
KTB = KT + 2         # + two ones/badd bias tiles for q,k (a DoubleRow pair)
GCOLS = HG * HD      # 1024 columns per group per projection
EPS = 1e-6
N_CORES = 8


def build_nc(has_norm_w: bool):
    nc = bacc.Bacc("TRN2", target_bir_lowering=False, debug=False,
                   enable_asserts=True, num_devices=N_CORES)

    x = nc.dram_tensor("x", [S, D], BF16, kind="ExternalInput").ap()
    cos = nc.dram_tensor("cos", [S, RD // 2], BF16, kind="ExternalInput").ap()
    sin = nc.dram_tensor("sin", [S, RD // 2], BF16, kind="ExternalInput").ap()
    wq = nc.dram_tensor("wq", [KTB * 128, GCOLS], F8, kind="ExternalInput").ap()
    wk = nc.dram_tensor("wk", [KTB * 128, GCOLS], F8, kind="ExternalInput").ap()
    wvh = nc.dram_tensor("wvh", [KTB * 128, GCOLS], F8, kind="ExternalInput").ap()
    wvl = nc.dram_tensor("wvl", [KTB * 128, GCOLS], F8, kind="ExternalInput").ap()
    # wo pre-laid-out on host as an fp8 hi/lo pair:
    # wo[p, m, hl, kb, c] = hi/lo split of wo[kb*128+p, m*128+c] * WS2
    wo = nc.dram_tensor("wo", [128, KT, 2, HG, 128], F8,
                        kind="ExternalInput").ap()
    gate = nc.dram_tensor("gate", [128, KT], F32, kind="ExternalInput").ap()
    vb = nc.dram_tensor("vb", [128, KT], F32, kind="ExternalInput").ap()
    if has_norm_w:
        wqn = nc.dram_tensor("wqn", [HD], BF16, kind="ExternalInput").ap()
        wkn = nc.dram_tensor("wkn", [HD], BF16, kind="ExternalInput").ap()
    # bf16 output halves the store DMA; the host gather sums the two
    # head-group partials in f32 (bf16 partial rounding ~1e-4 of the answer)
    out_t = nc.dram_tensor("out_t", [D, S], BF16, kind="ExternalOutput").ap()

    with tile.TileContext(nc) as tc:
        # ======== LEFT stack bottom: small persistent constants ====================
        misc_cm = tc.tile_pool(name="misc", bufs=1, side="left")
        misc = misc_cm.__enter__()
        ident_bf = misc.tile([128, 128], BF16)
        eps_t = misc.tile([128, 1], F32)
        eps128_t = misc.tile([128, 1], F32)
        # full-width fp8 ones: the DoubleRow denominator matmul then emits the
        # softmax sum replicated across all 128 partitions (same cost — matmul
        # time only scales with the moving free size), so no broadcast needed
        ones8 = misc.tile([128, 2, 128], F8)
        negc_t = misc.tile([128, 1], F32)
        ones_row = misc.tile([1, 128], BF16)

        def init_misc():
            # emitted after the first x DMAs so the load starts immediately
            make_identity(nc, ident_bf)
            nc.vector.memset(eps_t, EPS)
            nc.vector.memset(eps128_t, HD * EPS)
            nc.vector.memset(ones8, 1.0)
            nc.vector.memset(negc_t, -C_EXP)
            nc.vector.memset(ones_row, 1.0)
        gate_sb = misc.tile([128, KT], F32)
        vb_sb = misc.tile([128, KT], F32)
        rrk_all = misc.tile([128, NT, HG], F32)   # scaled k-rms reciprocals
        if has_norm_w:
            wqn_b = misc.tile([128, HG, HD], BF16)
            wkn_b = misc.tile([128, HG, HD], BF16)
        cos_all = misc.tile([128, NT, RD // 2], BF16)
        sin_all = misc.tile([128, NT, RD // 2], BF16)
        cs_tiles = [(cos_all[:, m, :], sin_all[:, m, :]) for m in range(NT)]

        # ======== RIGHT stack: natural-layout q/k (bf16) and v hi/lo (fp8) =========
        v_cm = tc.tile_pool(name="vpool", bufs=1, side="right")
        v_p = v_cm.__enter__()
        v2_hi = v_p.tile([128, NT, GCOLS], F8)
        v2_lo = v_p.tile([128, NT, GCOLS], F8)
        vtmp = v_p.tile([128, NT, GCOLS], BF16)
        natqk_cm = tc.tile_pool(name="natqk", bufs=1, side="right")
        natqk = natqk_cm.__enter__()
        qnat = natqk.tile([128, NT, GCOLS], BF16)
        knat = natqk.tile([128, NT, GCOLS], BF16)

        # resident qkv weights, triple-buffered: wv_hi/wv_lo both arrive
        # during B-k (tile 3 reuses tile 0's space, freed after B-q)
        wres_cm = tc.tile_pool(name="wres", bufs=3, side="right")
        wres_p = wres_cm.__enter__()
        wres_t = [wres_p.tile([128, KTB, GCOLS], F8, tag="wres", name=f"wres{pi}")
                  for pi in range(4)]
        wq_r = wq.rearrange("(k p) c -> p k c", p=128)
        wk_r = wk.rearrange("(k p) c -> p k c", p=128)
        wvh_r = wvh.rearrange("(k p) c -> p k c", p=128)
        wvl_r = wvl.rearrange("(k p) c -> p k c", p=128)
        w_drams = (wq_r, wk_r, wvh_r, wvl_r)

        def load_wres(pi, chunks=range(4)):
            for c4 in chunks:
                lo = c4 * 4
                hi = KTB if c4 == 3 else lo + 4   # last chunk carries tile 16
                nc.sync.dma_start(out=wres_t[pi][:, lo:hi, :],
                                  in_=w_drams[pi][:, lo:hi, :])

        # ======== phase A pools ====================================================
        xnT_cm = tc.tile_pool(name="xnT", bufs=1, side="left")
        xnT_p = xnT_cm.__enter__()
        xnT = xnT_p.tile([128, KTB, S], F8)  # [d_in_tile, d_tile, s]
        # bias-tile pair: tile16 = ones (x badd row), tile17 = 16*nmr per
        # column (x colsum(W)/16 row) — the mean-subtraction term
        # (-mean*rstd*colsum) is rank-1 and rides the same DoubleRow pair
        nc.gpsimd.memset(xnT[:, KT, :], 1.0)

        a_cm = tc.tile_pool(name="phA", bufs=3, side="left")
        a_p = a_cm.__enter__()
        a_small_cm = tc.tile_pool(name="phA_small", bufs=6, side="left")
        a_small = a_small_cm.__enter__()
        pst_cm = tc.tile_pool(name="ps_tr", bufs=4, space="PSUM")
        pst = pst_cm.__enter__()
        # pt: 4 bufs x 1 bank; ptn: 1 buf -> 5 banks + ps_qkv's 3 = 8

        # phase C pools opened early so later emission can overlap B on DVE
        c_cm = tc.tile_pool(name="phC", bufs=2, side="left")
        c_p = c_cm.__enter__()
        c_small_cm = tc.tile_pool(name="phC_small", bufs=2, side="left")
        c_small = c_small_cm.__enter__()

        psb_cm = tc.tile_pool(name="ps_qkv", bufs=3, space="PSUM")
        psb = psb_cm.__enter__()

        # ---- emit helpers ----
        def emit_A_dma(i):
            xt = a_p.tile([128, D], BF16, tag="xt", name=f"xt{i}")
            nc.sync.dma_start(out=xt[:, 0:D // 2],
                              in_=x[i * 128:(i + 1) * 128, 0:D // 2])
            nc.sync.dma_start(out=xt[:, D // 2:D],
                              in_=x[i * 128:(i + 1) * 128, D // 2:D])
            return xt

        def emit_A_stats(i, xt=None):
            """x load + LN statistics; emitted 2 s-tiles ahead of the body so
            the serial stats chain never gates the PE transposes."""
            if xt is None:
                xt = emit_A_dma(i)
            stats = a_small.tile([128, 4, 6], F32, tag="stats",
                                 name=f"stats{i}")
            xv = xt.rearrange("p (c f) -> p c f", c=4)
            for c in range(4):
                nc.vector.bn_stats(out=stats[:, c, :], in_=xv[:, c, :])
            mv = a_small.tile([128, 2], F32, tag="mv", name=f"mv{i}")
            nc.vector.bn_aggr(out=mv, in_=stats)
            rstd = a_small.tile([128, 1], F32, tag="rstd", name=f"rstd{i}")
            nc.scalar.activation(out=rstd, in_=mv[:, 1:2], func=AF.Sqrt,
                                 bias=eps_t, scale=1.0)
            nc.vector.reciprocal(out=rstd, in_=rstd)
            nmr = a_small.tile([128, 1], F32, tag="nmr", name=f"nmr{i}")
            nc.vector.tensor_scalar(out=nmr, in0=mv[:, 0:1], scalar1=rstd,
                                    scalar2=-1.0, op0=ALU.mult, op1=ALU.mult)
            # LN-apply rides the transpose matmul: rhs = diag(rstd); the
            # -mean*rstd term lands in xnT's 17th column tile (16*nmr in fp8,
            # contracted with host colsum(W)/16 rows by the bias DR pair)
            diag_t = a_small.tile([128, 128], BF16, tag="diag",
                                  name=f"diag{i}")
            nc.vector.tensor_scalar_mul(out=diag_t, in0=ident_bf,
                                        scalar1=rstd)
            nmr_bf = a_small.tile([128, 1], BF16, tag="nmr_bf",
                                  name=f"nmr_bf{i}")
            nc.vector.tensor_scalar_mul(out=nmr_bf, in0=nmr, scalar1=16.0)
            return xt, diag_t, nmr_bf

        def emit_A_bias(i, nmr_bf):
            # emitted a beat after the stats so the PE's ptn transpose never
            # waits on the (2-ahead) DVE stats chain
            ptn = pst.tile([1, 128], BF16, tag="ptn", name=f"ptn{i}", bufs=1)
            nc.tensor.transpose(ptn, nmr_bf, ident_bf)
            nmr_row = a_small.tile([1, 128], BF16, tag="nmr_row",
                                   name=f"nmr_row{i}")
            nc.vector.tensor_copy(out=nmr_row, in_=ptn)
            # partition-broadcast via a rank-1 PE matmul (nothing else can
            # replicate a row across partitions) into a pt-ring slot
            pbc = pst.tile([128, 4, 128], F32, tag="pt", name=f"pbc{i}")
            nc.tensor.matmul(pbc[:, 0, :], ones_row, nmr_row,
                             start=True, stop=True)
            nc.vector.tensor_copy(out=xnT[:, KT + 1, i * 128:(i + 1) * 128],
                                  in_=pbc[:, 0, :])

        def emit_A_body(i, xt, diag_t, nmr_bf):
            emit_A_bias(i, nmr_bf)
            # 4 normalize-transposes per psum tile -> one wide plain-copy
            # evac (modulation lives in the host-folded weights)
            for k4 in range(KT // 4):
                pt = pst.tile([128, 4, 128], F32, tag="pt")
                for j in range(4):
                    k = k4 * 4 + j
                    # j=0's start zeroes the whole 2KB PSUM zero-region; a
                    # start on later j would re-mark earlier slices pending-
                    # zero for the region-spanning group
                    nc.tensor.matmul(pt[:, j, :], xt[:, k * 128:(k + 1) * 128],
                                     diag_t, start=(j == 0), stop=(j == 3),
                                     skip_group_check=True)
                dst = xnT[:, k4 * 4:k4 * 4 + 4, i * 128:(i + 1) * 128]
                nc.scalar.copy(out=dst, in_=pt)

        def emit_Bm_qk(pi, nat, m, evac_engines):
            # last DoubleRow pair is the two ones-tiles x replicated badd/256
            # rows, adding the modulation/bias term
            wt = wres_t[pi]
            for n in range(2):
                ps = psb.tile([128, 512], F32, tag="ps")
                for k in range(0, KTB, 2):
                    nc.tensor.matmul(ps, xnT[:, k:k + 2, m * 128:(m + 1) * 128],
                                     wt[:, k:k + 2, n * 512:(n + 1) * 512],
                                     start=(k == 0), stop=(k == KTB - 2),
                                     perf_mode=DR)
                dst = nat[:, m, n * 512:(n + 1) * 512]
                if evac_engines[n] == "act":
                    nc.scalar.activation(out=dst, in_=ps, func=AF.Identity,
                                         scale=1.0 / WS)
                else:
                    nc.vector.tensor_scalar_mul(out=dst, in0=ps,
                                                scalar1=1.0 / WS)

        def emit_Bm_v(m):
            # v = xn @ (wv_hi + wv_lo), both chains in one PSUM group; evac
            # to bf16 vtmp (ACT), then fp8 hi (ACT); the lo split is deferred
            # to the D/E boundary where DVE has slack.
            for n in range(2):
                ps = psb.tile([128, 512], F32, tag="ps")
                for k in range(0, KT, 2):
                    nc.tensor.matmul(ps, xnT[:, k:k + 2, m * 128:(m + 1) * 128],
                                     wres_t[2][:, k:k + 2, n * 512:(n + 1) * 512],
                                     start=(k == 0), stop=False, perf_mode=DR)
                    nc.tensor.matmul(ps, xnT[:, k:k + 2, m * 128:(m + 1) * 128],
                                     wres_t[3][:, k:k + 2, n * 512:(n + 1) * 512],
                                     start=False, stop=False, perf_mode=DR)
                # bias pair for the -mean*rstd*colsum(wv) term (badd_v itself
                # folds into vb via sum(softmax)=1)
                nc.tensor.matmul(ps, xnT[:, KT:KT + 2, m * 128:(m + 1) * 128],
                                 wres_t[2][:, KT:KT + 2, n * 512:(n + 1) * 512],
                                 start=False, stop=True, perf_mode=DR)
                sl = slice(n * 512, (n + 1) * 512)
                nc.scalar.activation(out=vtmp[:, m, sl], in_=ps,
                                     func=AF.Identity, scale=1.0 / WS)
            nc.scalar.copy(out=v2_hi[:, m, :], in_=vtmp[:, m, :])

        def emit_Cq_stats(m):
            # emitted one m ahead of the rope half so the ACT sqrt never
            # blocks the next B evacuations queued behind it
            qm = qnat[:, m, :]
            sq = c_p.tile([128, GCOLS], BF16, tag="sqk")
            nc.gpsimd.tensor_mul(out=sq, in0=qm, in1=qm)
            ssq = c_small.tile([128, HG], BF16, tag="ssq")
            with nc.allow_low_precision(reason="bf16 rms stats; ~0.3% scale"):
                nc.vector.reduce_sum(out=ssq,
                                     in_=sq.rearrange("p (h d) -> p h d", h=HG),
                                     axis=mybir.AxisListType.X)
            rrq = c_small.tile([128, HG], BF16, tag="rrq")
            nc.scalar.activation(out=rrq, in_=ssq, func=AF.Sqrt,
                                 bias=eps_t, scale=1.0 / HD)
            with nc.allow_low_precision(reason="bf16 1/rms scale; 0.4% rel"):
                nc.vector.reciprocal(out=rrq, in_=rrq)
            return rrq

        def emit_Cq_rope(m, rrq):
            qm = qnat[:, m, :]
            qmh = qm.rearrange("p (h c) -> p h c", h=HG)
            (ct, st) = cs_tiles[m]
            cb = ct.unsqueeze(1).broadcast_to([128, HG, RD // 2])
            sb_ = st.unsqueeze(1).broadcast_to([128, HG, RD // 2])
            if has_norm_w:
                nc.vector.tensor_mul(out=qmh, in0=qmh, in1=wqn_b)
            _emit_rope(qmh, cb, sb_, pool_half=True)
            rrq_b = rrq.unsqueeze(2).broadcast_to([128, HG, HD])
            nc.vector.tensor_mul(out=qmh, in0=qmh, in1=rrq_b)

        def emit_Ck_stats(m):
            km = knat[:, m, :]
            sk_ = c_p.tile([128, GCOLS], BF16, tag="sqk")
            # square on the idle gpsimd engine: DVE is the pacer here
            nc.gpsimd.tensor_mul(out=sk_, in0=km, in1=km)
            ssk = c_small.tile([128, HG], BF16, tag="ssk")
            with nc.allow_low_precision(reason="bf16 rms stats; ~0.3% scale"):
                nc.vector.reduce_sum(out=ssk,
                                     in_=sk_.rearrange("p (h d) -> p h d", h=HG),
                                     axis=mybir.AxisListType.X)
            nc.scalar.activation(out=rrk_all[:, m, :], in_=ssk, func=AF.Sqrt,
                                 bias=eps128_t, scale=1.0)
            nc.vector.reciprocal(out=rrk_all[:, m, :], in_=rrk_all[:, m, :])

        def emit_Ck_rope(m):
            km = knat[:, m, :]
            kmh = km.rearrange("p (h c) -> p h c", h=HG)
            (ct, st) = cs_tiles[m]
            cb = ct.unsqueeze(1).broadcast_to([128, HG, RD // 2])
            sb_ = st.unsqueeze(1).broadcast_to([128, HG, RD // 2])
            if has_norm_w:
                nc.vector.tensor_mul(out=kmh, in0=kmh, in1=wkn_b)
            _emit_rope(kmh, cb, sb_, pool_half=False)

        def _emit_rope(mh, cb, sb_, pool_half):
            # host-de-interleaved: cols [0:32]=x0, [32:64]=x1 per head.
            # pool_half routes the y1 branch to gpsimd (free in the q region).
            x0 = mh[:, :, 0:RD // 2]
            x1 = mh[:, :, RD // 2:RD]
            t0 = c_small.tile([128, HG, RD // 2], BF16, tag="t0")
            t1 = c_small.tile([128, HG, RD // 2], BF16, tag="t1")
            t2 = c_small.tile([128, HG, RD // 2], BF16, tag="t2")
            t3 = c_small.tile([128, HG, RD // 2], BF16, tag="t3")
            e = nc.gpsimd if pool_half else nc.vector
            nc.vector.tensor_mul(out=t0, in0=x0, in1=cb)
            nc.vector.tensor_mul(out=t1, in0=x1, in1=sb_)
            e.tensor_mul(out=t2, in0=x0, in1=sb_)
            e.tensor_mul(out=t3, in0=x1, in1=cb)
            nc.vector.tensor_sub(out=x0, in0=t0, in1=t1)
            e.tensor_add(out=x1, in0=t2, in1=t3)

        # ---- fused A + B-q emission ----
        xt01 = {i: emit_A_dma(i) for i in (0, 1)}
        init_misc()
        pend = {i: emit_A_stats(i, xt01.pop(i)) for i in (0, 1)}
        for i in range(NT):
            if i + 2 < NT:
                pend[i + 2] = emit_A_stats(i + 2)
            emit_A_body(i, *pend.pop(i))
            # all wq chunks MUST be emitted before the first emit_Bm below
            # (deps only order reads after already-emitted writes)
            if i == 0:
                load_wres(0, range(0, 2))
            elif i == 1:
                load_wres(0, range(2, 4))
            elif i == 3:
                load_wres(1)          # wk, consumed after A+B-q
            if i >= 1:
                emit_Bm_qk(0, qnat, i - 1, ("act", "act"))
        emit_Bm_qk(0, qnat, NT - 1, ("act", "act"))

        # deferred misc loads (consumed in phases C/E/F) — single batched DMAs
        # so their dispatch overhead doesn't contend with the x/w streams
        nc.sync.dma_start(out=gate_sb, in_=gate)
        nc.sync.dma_start(out=vb_sb, in_=vb)
        if has_norm_w:
            def _bcast_from_dram(ap, parts, reps=None):
                newap = [[0, parts]]
                if reps is not None:
                    newap.append([0, reps])
                newap += list(ap.ap)
                return bass.AP(tensor=ap.tensor, offset=ap.offset, ap=newap)
            nc.sync.dma_start(out=wqn_b, in_=_bcast_from_dram(wqn, 128, reps=HG))
            nc.sync.dma_start(out=wkn_b, in_=_bcast_from_dram(wkn, 128, reps=HG))
        nc.sync.dma_start(out=cos_all, in_=cos.rearrange("(m p) f -> p m f", p=128))
        nc.sync.dma_start(out=sin_all, in_=sin.rearrange("(m p) f -> p m f", p=128))

        # ---- B-k (PE) interleaved with C-q (DVE/Pool) per m-tile ----
        rrq_pend = {0: emit_Cq_stats(0)}
        for m in range(NT):
            emit_Bm_qk(1, knat, m, ("act", "act"))
            if m + 1 < NT:
                rrq_pend[m + 1] = emit_Cq_stats(m + 1)
            emit_Cq_rope(m, rrq_pend.pop(m))
            if m == 0:
                load_wres(2)          # wv_hi
                load_wres(3)          # wv_lo
        # ---- B-v (PE) interleaved with C-k (DVE/Pool) ----
        emit_Ck_stats(0)
        for m in range(NT):
            emit_Bm_v(m)
            if m + 1 < NT:
                emit_Ck_stats(m + 1)
            emit_Ck_rope(m)

        psb_cm.__exit__(None, None, None)
        pst_cm.__exit__(None, None, None)
        c_small_cm.__exit__(None, None, None)
        c_cm.__exit__(None, None, None)
        a_small_cm.__exit__(None, None, None)
        a_cm.__exit__(None, None, None)
        xnT_cm.__exit__(None, None, None)
        wres_cm.__exit__(None, None, None)

        # ======== phases D/E/F share the left stack ================================
        oT_cm = tc.tile_pool(name="oT", bufs=1, side="left")
        oT_p = oT_cm.__enter__()
        oT_hi = oT_p.tile([128, HG, S], F8)
        oT_lo = oT_p.tile([128, HG, S], F8)

        # wo prefetch (resident; consumed in phase F)
        wo_cm = tc.tile_pool(name="wo_res", bufs=1, side="left")
        wo_p = wo_cm.__enter__()
        wo_hi_sb = wo_p.tile([128, KT, HG, 128], F8)
        wo_lo_sb = wo_p.tile([128, KT, HG, 128], F8)

        # ---- phase D: transpose q, k -> [hd, s] per head (batched evacs)
        # q evacs on ACT (DVE is finishing C-k); k evacs on DVE; the v lo
        # split (SBUF-only, deferred from B-v) fills DVE alongside.
        qkT_cm = tc.tile_pool(name="qkT", bufs=1, side="left")
        qkT_p = qkT_cm.__enter__()
        qT = qkT_p.tile([128, HG, S], BF16)
        kT = qkT_p.tile([128, HG, S], BF16)
        pst2_cm = tc.tile_pool(name="ps_tr2", bufs=4, space="PSUM")
        pst2 = pst2_cm.__enter__()

        def emit_D(nat, dstT, h, evac):
            pt8 = pst2.tile([128, S], BF16, tag="pt8")
            for m in range(NT):
                nc.tensor.transpose(pt8[:, m * 128:(m + 1) * 128],
                                    nat[:, m, h * 128:(h + 1) * 128],
                                    ident_bf)
            dst = dstT[:, h, :]
            if evac == "act":
                nc.scalar.copy(out=dst, in_=pt8)
            else:
                nc.vector.tensor_copy(out=dst, in_=pt8)

        for h in range(HG):
            emit_D(qnat, qT, h, "act")
            if h % 2 == 0:
                m = h // 2 * 2
                with nc.allow_low_precision(reason="fp8 v lo split"):
                    nc.vector.tensor_sub(out=v2_lo[:, m, :],
                                         in0=vtmp[:, m, :], in1=v2_hi[:, m, :])
                    nc.vector.tensor_sub(out=v2_lo[:, m + 1, :],
                                         in0=vtmp[:, m + 1, :],
                                         in1=v2_hi[:, m + 1, :])
        for h in range(HG):
            emit_D(knat, kT, h, "dve")
        pst2_cm.__exit__(None, None, None)
        # (natqk stays open through E/F: closing it between D and E would
        # add a pool barrier; SBUF has room)

        # wo loads (consumed in F; DMA-engine time hidden under phase E)
        for m in range(KT):
            nc.sync.dma_start(out=wo_hi_sb[:, m, :, :], in_=wo[:, m, 0, :, :])
            nc.sync.dma_start(out=wo_lo_sb[:, m, :, :], in_=wo[:, m, 1, :, :])

        # ---- phase E: attention per head (fp8 DoubleRow attn*v) ----
        at2_cm = tc.tile_pool(name="at2", bufs=2, side="left")
        at2_p = at2_cm.__enter__()
        rs_cm = tc.tile_pool(name="rsb", bufs=2, side="left")
        rs_p = rs_cm.__enter__()
        pssc_cm = tc.tile_pool(name="ps_sc", bufs=2, space="PSUM")
        pssc = pssc_cm.__enter__()
        pso_cm = tc.tile_pool(name="ps_o", bufs=1, space="PSUM")
        pso = pso_cm.__enter__()
        psdn_cm = tc.tile_pool(name="ps_dn", bufs=2, space="PSUM")
        psdn = psdn_cm.__enter__()

        def emit_scores(h, m):
            sc = pssc.tile([128, S], F32, tag="sc")
            lhs_k = kT[:, h, m * 128:(m + 1) * 128]
            nc.tensor.matmul(sc[:, 0:512], lhs_k, qT[:, h, 0:512],
                             start=True, stop=True)
            nc.tensor.matmul(sc[:, 512:1024], lhs_k, qT[:, h, 512:1024],
                             start=True, stop=True)
            return sc

        for h in range(HG):
            o_ps = pso.tile([128, S], F32, tag="o_ps", name=f"o_ps{h}")
            dns = [psdn.tile([128, 512], F32, tag="dn", name=f"dn{h}_{g}")
                   for g in range(2)]
            sc_next = emit_scores(h, 0)
            at = None
            for m in range(NT):
                sc = sc_next
                sc_next = emit_scores(h, m + 1) if m + 1 < NT else None
                if m % 2 == 0:
                    at = at2_p.tile([128, 2, S], F8, tag="at", name="at")
                # fp8 attn weights: exp(score*rrk - C) stays under e4m3 240;
                # the e^-C factor cancels between numerator and denominator
                nc.scalar.activation(out=at[:, m % 2, :], in_=sc, func=AF.Exp,
                                     scale=rrk_all[:, m, h:h + 1], bias=negc_t)
                if m % 2 == 1:
                    mp = m // 2
                    first, last = (mp == 0), (mp == NT // 2 - 1)
                    for g in range(2):
                        # denominator on PE: ones^T @ at, replicated rows
                        nc.tensor.matmul(dns[g], ones8,
                                         at[:, :, g * 512:(g + 1) * 512],
                                         start=first, stop=last, perf_mode=DR)
                    for vi, vsrc in enumerate((v2_hi, v2_lo)):
                        v_mh = vsrc[:, 2 * mp:2 * mp + 2, h * 128:(h + 1) * 128]
                        nc.tensor.matmul(o_ps[:, 0:512], v_mh, at[:, :, 0:512],
                                         start=(first and vi == 0),
                                         stop=(last and vi == 1), perf_mode=DR)
                        nc.tensor.matmul(o_ps[:, 512:1024], v_mh,
                                         at[:, :, 512:1024],
                                         start=(first and vi == 0),
                                         stop=(last and vi == 1), perf_mode=DR)
            for g in range(2):
                sl = slice(g * 512, (g + 1) * 512)
                sums = rs_p.tile([128, 512], F32, tag="sums", name=f"sums{h}{g}")
                nc.vector.reciprocal(out=sums, in_=dns[g])
                obf = rs_p.tile([128, 512], BF16, tag="obf", name=f"obf{h}{g}")
                nc.vector.tensor_mul(out=obf, in0=o_ps[:, sl], in1=sums)
                nc.vector.tensor_copy(out=oT_hi[:, h, sl], in_=obf)
                with nc.allow_low_precision(reason="fp8 o lo split"):
                    nc.vector.tensor_sub(out=oT_lo[:, h, sl], in0=obf,
                                         in1=oT_hi[:, h, sl])

        # ---- phase F: out projection as three fp8 DoubleRow chains
        # (hi*hi + hi*lo + lo*hi; the dropped lo*lo term is ~0.1%^2), emitted
        # inside E's psum pools so no pool barrier splits E/F
        F_CHAINS = 3
        F_KP = HG // 2

        def emit_F_mm(po, m, ci, kp):
            wsrc, osrc = ((wo_hi_sb, oT_hi), (wo_hi_sb, oT_lo),
                          (wo_lo_sb, oT_hi))[ci]
            first = (ci == 0 and kp == 0)
            last = (ci == F_CHAINS - 1 and kp == F_KP - 1)
            lhs = wsrc[:, m, 2 * kp:2 * kp + 2, :]
            nc.tensor.matmul(po[:, 0:512], lhs, osrc[:, 2 * kp:2 * kp + 2, 0:512],
                             start=first, stop=last, perf_mode=DR)
            nc.tensor.matmul(po[:, 512:1024], lhs,
                             osrc[:, 2 * kp:2 * kp + 2, 512:1024],
                             start=first, stop=last, perf_mode=DR)

        def emit_F_evac(po, m):
            ot_t = rs_p.tile([128, S], BF16, tag="ot_t", name=f"ot_t{m}")
            if m == KT - 1:
                # the kernel's tail: halve the evac+store chain so the first
                # half's DMA overlaps the second half's evacuation
                for g in (0, 1):
                    sl = slice(g * 512, (g + 1) * 512)
                    nc.vector.tensor_scalar(out=ot_t[:, sl], in0=po[:, sl],
                                            scalar1=gate_sb[:, m:m + 1],
                                            scalar2=vb_sb[:, m:m + 1],
                                            op0=ALU.mult, op1=ALU.add)
                    nc.sync.dma_start(out=out_t[m * 128:(m + 1) * 128, sl],
                                      in_=ot_t[:, sl])
                return
            if m % 2 == 0:
                nc.scalar.activation(out=ot_t, in_=po, func=AF.Identity,
                                     bias=vb_sb[:, m:m + 1],
                                     scale=gate_sb[:, m:m + 1])
            else:
                nc.vector.tensor_scalar(out=ot_t, in0=po,
                                        scalar1=gate_sb[:, m:m + 1],
                                        scalar2=vb_sb[:, m:m + 1],
                                        op0=ALU.mult, op1=ALU.add)
            nc.sync.dma_start(out=out_t[m * 128:(m + 1) * 128, :], in_=ot_t)

        def alloc_po(m):
            # alternate between the two dead attention psum pools for slack
            return (pssc.tile([128, S], F32, tag="sc", name=f"po{m}")
                    if m % 2 == 0
                    else pso.tile([128, S], F32, tag="o_ps", name=f"po{m}"))

        # first two output tiles interleave their chain loops so F has dense
        # matmuls queued before it needs the last head's normalization
        steps = [(ci, kp) for ci in range(F_CHAINS) for kp in range(F_KP)]
        po01 = {m: alloc_po(m) for m in (0, 1)}
        for m in (0, 1):
            for (ci, kp) in steps[:-2]:
                emit_F_mm(po01[m], m, ci, kp)
        for m in (0, 1):
            for (ci, kp) in steps[-2:]:
                emit_F_mm(po01[m], m, ci, kp)
            emit_F_evac(po01[m], m)
        for m in range(2, KT):
            po = alloc_po(m)
            for (ci, kp) in steps:
                emit_F_mm(po, m, ci, kp)
            emit_F_evac(po, m)

        psdn_cm.__exit__(None, None, None)
        pso_cm.__exit__(None, None, None)
        pssc_cm.__exit__(None, None, None)
        rs_cm.__exit__(None, None, None)
        at2_cm.__exit__(None, None, None)
        qkT_cm.__exit__(None, None, None)
        natqk_cm.__exit__(None, None, None)
        v_cm.__exit__(None, None, None)
        wo_cm.__exit__(None, None, None)
        oT_cm.__exit__(None, None, None)
        misc_cm.__exit__(None, None, None)

    nc.compile()
    return nc


_NC_CACHE = {}


def _get_nc(has_norm_w):
    if has_norm_w not in _NC_CACHE:
        _NC_CACHE[has_norm_w] = build_nc(has_norm_w)
    return _NC_CACHE[has_norm_w]


# per-head column permutation de-interleaving rope pairs:
# [0,2,..,62, 1,3,..,63, 64..127] within each head's 128 columns
def _qk_perm():
    base = np.concatenate([np.arange(0, RD, 2), np.arange(1, RD, 2),
                           np.arange(RD, HD)])
    return np.concatenate([h * HD + base for h in range(HG)])


_PERM = _qk_perm()
F8NP = ml_dtypes.float8_e4m3


def prep_in_maps(x, mod, cos, sin, qkv_w, qkv_b, mod_w, mod_b, out_w, out_b,
                 norm_q_w, norm_k_w):
    """Host-side sharding. Returns (in_maps, flags, x_np)."""
    x = np.asarray(x, dtype=np.float32)
    m3 = np.asarray(mod, np.float32) @ np.asarray(mod_w, np.float32) \
        + np.asarray(mod_b, np.float32)
    bias, scale, gatef = np.split(m3, 3, axis=-1)          # [B, D] each
    scale1p = (1.0 + scale).astype(np.float32)

    qkv_b = np.asarray(qkv_b, np.float32)
    has_norm_w = not (np.allclose(norm_q_w, 1.0) and np.allclose(norm_k_w, 1.0))

    cosb = np.ascontiguousarray(np.asarray(cos, np.float32).astype(ml_dtypes.bfloat16))
    sinb = np.ascontiguousarray(np.asarray(sin, np.float32).astype(ml_dtypes.bfloat16))
    qkv_w = np.asarray(qkv_w, np.float32)
    out_w = np.asarray(out_w, np.float32)
    out_b = np.asarray(out_b, np.float32)

    # per-head rope-pair de-interleave permutation (shared by q and k; inner
    # products and rms are invariant; cos/sin indices line up with x0/x1 halves)
    perm = _PERM
    hd_perm = np.concatenate([np.arange(0, RD, 2), np.arange(1, RD, 2),
                              np.arange(RD, HD)])

    def with_bias_tile(w_scaled_f8, badd, colsum_scaled):
        # tile16 rows (x ones): badd*WS/128; tile17 rows (x 16*nmr):
        # colsum(W_fp8)*WS / (128*16) — the rank-1 mean-subtraction term
        brow = np.broadcast_to((badd * (WS / 128.0))[None, :],
                               (128, badd.shape[0]))
        crow = np.broadcast_to((colsum_scaled / 2048.0)[None, :],
                               (128, badd.shape[0]))
        return np.ascontiguousarray(np.concatenate(
            [w_scaled_f8, brow.astype(F8NP), crow.astype(F8NP)], axis=0))

    in_maps = []
    for c in range(N_CORES):
        b, g = divmod(c, 2)
        lo = g * GCOLS
        s1p = scale1p[b][:, None]                          # [D, 1]
        wq_c = (qkv_w[:, lo:lo + GCOLS] * s1p)[:, perm]
        wk_c = (qkv_w[:, 2048 + lo:2048 + lo + GCOLS] * s1p)[:, perm]
        wv_c = qkv_w[:, 4096 + lo:4096 + lo + GCOLS] * s1p
        # modulation/qkv bias rows: b @ W (+ qkv bias), per projection
        badd_q = bias[b] @ qkv_w[:, lo:lo + GCOLS] + qkv_b[lo:lo + GCOLS]
        badd_k = bias[b] @ qkv_w[:, 2048 + lo:2048 + lo + GCOLS] \
            + qkv_b[2048 + lo:2048 + lo + GCOLS]
        badd_v = bias[b] @ qkv_w[:, 4096 + lo:4096 + lo + GCOLS] \
            + qkv_b[4096 + lo:4096 + lo + GCOLS]
        wo_c = out_w[lo:lo + GCOLS, :]                      # [GCOLS, D]
        # wo_l[p, m, kb, c] = wo_c[kb*128+p, m*128+c], fp8 hi/lo pair
        wo_l = np.transpose(
            wo_c.reshape(HG, 128, KT, 128), (1, 2, 0, 3)) * WS2
        wo_hi = wo_l.astype(F8NP)
        wo_lo = (wo_l - wo_hi.astype(np.float32)).astype(F8NP)
        wo_pair = np.stack([wo_hi, wo_lo], axis=2)   # [128, KT, 2, HG, 128]
        # v weights as an fp8 hi+lo pair at a shared scale: lo lands in the
        # e4m3 subnormal range, leaving ~0.1% net weight error
        wvh_c = (wv_c * WS).astype(F8NP)
        wvl_c = (wv_c * WS - wvh_c.astype(np.float32)).astype(F8NP)
        zrow = np.zeros((256, GCOLS), np.float32)
        wq_f8 = (wq_c * WS).astype(F8NP)
        wk_f8 = (wk_c * WS).astype(F8NP)
        # v bias folds into the output bias: sum(softmax)=1 makes o shift by
        # badd_v exactly, i.e. the final out shifts by (badd_v @ wo) * gate
        vb_full = badd_v @ wo_c                             # [D]
        if g == 0:
            vb_full = vb_full + out_b
        vbf = (vb_full * gatef[b]).astype(np.float32)
        im = {
            "x": np.ascontiguousarray(x[b].astype(ml_dtypes.bfloat16)),
            "cos": cosb, "sin": sinb,
            "wq": with_bias_tile(wq_f8, badd_q[perm],
                                 wq_f8.astype(np.float32).sum(0)),
            "wk": with_bias_tile(wk_f8, badd_k[perm],
                                 wk_f8.astype(np.float32).sum(0)),
            "wvh": with_bias_tile(wvh_c, np.zeros(GCOLS, np.float32),
                                  wvh_c.astype(np.float32).sum(0)
                                  + wvl_c.astype(np.float32).sum(0)),
            "wvl": np.ascontiguousarray(
                np.concatenate([wvl_c, zrow.astype(F8NP)], axis=0)),
            "wo": np.ascontiguousarray(wo_pair),
            "gate": np.ascontiguousarray(
                (gatef[b] / WS2).reshape(KT, 128).T),
            "vb": np.ascontiguousarray(vbf.reshape(KT, 128).T),
        }
        if has_norm_w:
            im["wqn"] = np.ascontiguousarray(
                np.asarray(norm_q_w, np.float32)[hd_perm].astype(ml_dtypes.bfloat16))
            im["wkn"] = np.ascontiguousarray(
                np.asarray(norm_k_w, np.float32)[hd_perm].astype(ml_dtypes.bfloat16))
        in_maps.append(im)
    return in_maps, has_norm_w, x


def gather(results, x):
    B = x.shape[0]
    outs = []
    for b in range(B):
        p = results[2 * b]["out_t"].astype(np.float32) \
            + results[2 * b + 1]["out_t"].astype(np.float32)        # [D, S]
        outs.append(p.T + x[b])
    return np.stack(outs).astype(np.float32)


def kernel(**inputs) -> np.ndarray:
    in_maps, has_norm_w, x = prep_in_maps(**inputs)
    nc = _get_nc(has_norm_w)
    res = run_bass_kernel_spmd(nc, in_maps, core_ids=list(range(N_CORES)))
    return gather(res.results, x)


if __name__ == "__main__":
    import time
    t0 = time.time()
    nc = build_nc(False)
    print("build+compile ok in", time.time() - t0, "s")


# revision 75
# speedup vs baseline: 1.1858x; 1.0399x over previous
"""Trainium2 Bass kernel for ModalityAttention (B=4, S=1024, D=2048, H=16, HD=128, RD=64).

Sharding: 8 cores = 4 batches x 2 head-groups (8 heads each).
Each core computes, for its (batch b, head-group g):
  layernorm(x[b]) -> qkv projection (modulation host-folded into the
  weights) for its 8 heads -> rmsnorm + rope -> attention
  -> partial out-projection (transposed layout) with gate folded in.
Host gathers: out[b] = (partial_g0 + partial_g1).T + x[b]

Modulation fold: qkv(xmod) with xmod=(1+s)*xn+b equals
xn @ ((1+s)[d] * W) + b @ W.  The row-scale goes into the host fp8
weight cast (per-core batch); the additive row b@W rides the matmul as
a 17th contraction tile (fp8 ones in xnT x replicated badd/128 rows)
for q/k, and for v folds into the output bias via sum(softmax)=1:
adding c[col] to v shifts o by exactly c, i.e. out by (c @ wo) -- a
host-side vb adjustment.  qkv_b folds into the same terms.

Precision strategy (fp8 e4m3 DoubleRow matmuls where softmax/rms
renormalization absorbs the quantization noise, near-exact elsewhere):
  - q,k projections: fp8 DoubleRow (xn fp8, weights fp8*256).
  - v projection: fp8 DoubleRow with the weight split into hi+lo fp8
    at a shared scale, accumulating in one PSUM group -> ~0.1% weight
    error (the v path feeds the output linearly, it cannot absorb noise).
  - attn*v: fp8 DoubleRow with v split hi+lo on device; exp emits fp8
    with a constant -C_EXP bias (cancels in the softmax ratio) to stay
    under the e4m3 +-240 overflow.
  - softmax denominator: ones-vector DoubleRow matmul on PE whose
    replicated output rows make the broadcast free.
  - scores and out projection: bf16.

Engine balance: gpsimd (Pool) takes SBUF-only elementwise work (k
squares, part of q rope) since it cannot touch PSUM; evacs batch 4
transposes per PSUM tile to amortize the fp8 no-2x penalty on DVE/ACT.
"""
import os, sys

for _p in ("/opt/trn_rl_repo", "/root/.axon_site/_ro/trn_rl_repo", "/root/.axon_site"):
    if os.path.isdir(_p) and _p not in sys.path:
        sys.path.insert(0, _p)

import numpy as np
import ml_dtypes
import concourse.bass as bass
import concourse.bacc as bacc
import concourse.mybir as mybir
import concourse.tile as tile
from concourse import bass_isa
from concourse.masks import make_identity
from concourse.bass_utils import run_bass_kernel_spmd

F32 = mybir.dt.float32
BF16 = mybir.dt.bfloat16
F8 = mybir.dt.float8e4
DR = mybir.MatmulPerfMode.DoubleRow
AF = mybir.ActivationFunctionType
ALU = mybir.AluOpType
WS = 256.0      # host-side qkv weight scale (fp8 e4m3 subnormal floor)
WS2 = 256.0     # host-side out-proj weight scale (descale rides the gate)
C_EXP = 2.0     # exp(score - C_EXP): keeps fp8 attn weights under 240
S, D, HG, HD, RD = 1024, 2048, 8, 128, 64
NT = S // 128        # 8 s-tiles
KT = D // 128        # 16 d-tiles
KTB = KT + 2         # + two ones/badd bias tiles for q,k (a DoubleRow pair)
GCOLS = HG * HD      # 1024 columns per group per projection
EPS = 1e-6
N_CORES = 8


def build_nc(has_norm_w: bool):
    nc = bacc.Bacc("TRN2", target_bir_lowering=False, debug=False,
                   enable_asserts=True, num_devices=N_CORES)

    x = nc.dram_tensor("x", [S, D], BF16, kind="ExternalInput").ap()
    cos = nc.dram_tensor("cos", [S, RD // 2], BF16, kind="ExternalInput").ap()
    sin = nc.dram_tensor("sin", [S, RD // 2], BF16, kind="ExternalInput").ap()
    wq = nc.dram_tensor("wq", [KTB * 128, GCOLS], F8, kind="ExternalInput").ap()
    wk = nc.dram_tensor("wk", [KTB * 128, GCOLS], F8, kind="ExternalInput").ap()
    wvh = nc.dram_tensor("wvh", [KTB * 128, GCOLS], F8, kind="ExternalInput").ap()
    wvl = nc.dram_tensor("wvl", [KTB * 128, GCOLS], F8, kind="ExternalInput").ap()
    # wo pre-laid-out on host as an fp8 hi/lo pair:
    # wo[p, m, hl, kb, c] = hi/lo split of wo[kb*128+p, m*128+c] * WS2
    wo = nc.dram_tensor("wo", [128, KT, 2, HG, 128], F8,
                        kind="ExternalInput").ap()
    gate = nc.dram_tensor("gate", [128, KT], F32, kind="ExternalInput").ap()
    vb = nc.dram_tensor("vb", [128, KT], F32, kind="ExternalInput").ap()
    if has_norm_w:
        wqn = nc.dram_tensor("wqn", [HD], BF16, kind="ExternalInput").ap()
        wkn = nc.dram_tensor("wkn", [HD], BF16, kind="ExternalInput").ap()
    # bf16 output halves the store DMA; the host gather sums the two
    # head-group partials in f32 (bf16 partial rounding ~1e-4 of the answer)
    out_t = nc.dram_tensor("out_t", [D, S], BF16, kind="ExternalOutput").ap()

    with tile.TileContext(nc) as tc:
        # ======== LEFT stack bottom: small persistent constants ====================
        misc_cm = tc.tile_pool(name="misc", bufs=1, side="left")
        misc = misc_cm.__enter__()
        ident_bf = misc.tile([128, 128], BF16)
        eps_t = misc.tile([128, 1], F32)
        eps128_t = misc.tile([128, 1], F32)
        # full-width fp8 ones: the DoubleRow denominator matmul then emits the
        # softmax sum replicated across all 128 partitions (same cost — matmul
        # time only scales with the moving free size), so no broadcast needed
        ones8 = misc.tile([128, 2, 128], F8)
        negc_t = misc.tile([128, 1], F32)
        ones_row = misc.tile([1, 128], BF16)

        def init_misc():
            # emitted after the first x DMAs so the load starts immediately;
            # memsets ride gpsimd to keep the DVE queue clear for LN stats
            make_identity(nc, ident_bf)
            nc.gpsimd.memset(eps_t, EPS)
            nc.gpsimd.memset(eps128_t, HD * EPS)
            nc.gpsimd.memset(ones8, 1.0)
            nc.gpsimd.memset(negc_t, -C_EXP)
            nc.gpsimd.memset(ones_row, 1.0)
        gate_sb = misc.tile([128, KT], F32)
        vb_sb = misc.tile([128, KT], F32)
        rrk_all = misc.tile([128, NT, HG], F32)   # scaled k-rms reciprocals
        if has_norm_w:
            wqn_b = misc.tile([128, HG, HD], BF16)
            wkn_b = misc.tile([128, HG, HD], BF16)
        cos_all = misc.tile([128, NT, RD // 2], BF16)
        sin_all = misc.tile([128, NT, RD // 2], BF16)
        cs_tiles = [(cos_all[:, m, :], sin_all[:, m, :]) for m in range(NT)]

        # ======== RIGHT stack: natural-layout q/k (bf16) and v hi/lo (fp8) =========
        v_cm = tc.tile_pool(name="vpool", bufs=1, side="right")
        v_p = v_cm.__enter__()
        v2_hi = v_p.tile([128, NT, GCOLS], F8)
        v2_lo = v_p.tile([128, NT, GCOLS], F8)
        vtmp = v_p.tile([128, NT, GCOLS], BF16)
        natqk_cm = tc.tile_pool(name="natqk", bufs=1, side="right")
        natqk = natqk_cm.__enter__()
        qnat = natqk.tile([128, NT, GCOLS], BF16)
        knat = natqk.tile([128, NT, GCOLS], BF16)

        # resident qkv weights, triple-buffered: wv_hi/wv_lo both arrive
        # during B-k (tile 3 reuses tile 0's space, freed after B-q)
        wres_cm = tc.tile_pool(name="wres", bufs=3, side="right")
        wres_p = wres_cm.__enter__()
        wres_t = [wres_p.tile([128, KTB, GCOLS], F8, tag="wres", name=f"wres{pi}")
                  for pi in range(4)]
        wq_r = wq.rearrange("(k p) c -> p k c", p=128)
        wk_r = wk.rearrange("(k p) c -> p k c", p=128)
        wvh_r = wvh.rearrange("(k p) c -> p k c", p=128)
        wvl_r = wvl.rearrange("(k p) c -> p k c", p=128)
        w_drams = (wq_r, wk_r, wvh_r, wvl_r)

        def load_wres(pi, chunks=range(4)):
            for c4 in chunks:
                lo = c4 * 4
                hi = KTB if c4 == 3 else lo + 4   # last chunk carries tile 16
                nc.sync.dma_start(out=wres_t[pi][:, lo:hi, :],
                                  in_=w_drams[pi][:, lo:hi, :])

        # ======== phase A pools ====================================================
        xnT_cm = tc.tile_pool(name="xnT", bufs=1, side="left")
        xnT_p = xnT_cm.__enter__()
        xnT = xnT_p.tile([128, KTB, S], F8)  # [d_in_tile, d_tile, s]
        # bias-tile pair: tile16 = ones (x badd row), tile17 = 16*nmr per
        # column (x colsum(W)/16 row) — the mean-subtraction term
        # (-mean*rstd*colsum) is rank-1 and rides the same DoubleRow pair
        nc.gpsimd.memset(xnT[:, KT, :], 1.0)

        a_cm = tc.tile_pool(name="phA", bufs=3, side="left")
        a_p = a_cm.__enter__()
        a_small_cm = tc.tile_pool(name="phA_small", bufs=6, side="left")
        a_small = a_small_cm.__enter__()
        pst_cm = tc.tile_pool(name="ps_tr", bufs=4, space="PSUM")
        pst = pst_cm.__enter__()
        # pt: 4 bufs x 1 bank; ptn: 1 buf -> 5 banks + ps_qkv's 3 = 8

        # phase C pools opened early so later emission can overlap B on DVE
        c_cm = tc.tile_pool(name="phC", bufs=2, side="left")
        c_p = c_cm.__enter__()
        c_small_cm = tc.tile_pool(name="phC_small", bufs=2, side="left")
        c_small = c_small_cm.__enter__()

        psb_cm = tc.tile_pool(name="ps_qkv", bufs=3, space="PSUM")
        psb = psb_cm.__enter__()

        # ---- emit helpers ----
        def emit_A_dma(i):
            xt = a_p.tile([128, D], BF16, tag="xt", name=f"xt{i}")
            nc.sync.dma_start(out=xt[:, 0:D // 2],
                              in_=x[i * 128:(i + 1) * 128, 0:D // 2])
            nc.sync.dma_start(out=xt[:, D // 2:D],
                              in_=x[i * 128:(i + 1) * 128, D // 2:D])
            return xt

        def emit_A_stats(i, xt=None):
            """x load + LN statistics; emitted 2 s-tiles ahead of the body so
            the serial stats chain never gates the PE transposes."""
            if xt is None:
                xt = emit_A_dma(i)
            stats = a_small.tile([128, 4, 6], F32, tag="stats",
                                 name=f"stats{i}")
            xv = xt.rearrange("p (c f) -> p c f", c=4)
            for c in range(4):
                nc.vector.bn_stats(out=stats[:, c, :], in_=xv[:, c, :])
            mv = a_small.tile([128, 2], F32, tag="mv", name=f"mv{i}")
            nc.vector.bn_aggr(out=mv, in_=stats)
            rstd = a_small.tile([128, 1], F32, tag="rstd", name=f"rstd{i}")
            nc.scalar.activation(out=rstd, in_=mv[:, 1:2], func=AF.Sqrt,
                                 bias=eps_t, scale=1.0)
            nc.vector.reciprocal(out=rstd, in_=rstd)
            nmr = a_small.tile([128, 1], F32, tag="nmr", name=f"nmr{i}")
            nc.vector.tensor_scalar(out=nmr, in0=mv[:, 0:1], scalar1=rstd,
                                    scalar2=-1.0, op0=ALU.mult, op1=ALU.mult)
            # LN-apply rides the transpose matmul: rhs = diag(rstd); the
            # -mean*rstd term lands in xnT's 17th column tile (16*nmr in fp8,
            # contracted with host colsum(W)/16 rows by the bias DR pair)
            diag_t = a_small.tile([128, 128], BF16, tag="diag",
                                  name=f"diag{i}")
            nc.vector.tensor_scalar_mul(out=diag_t, in0=ident_bf,
                                        scalar1=rstd)
            nmr_bf = a_small.tile([128, 1], BF16, tag="nmr_bf",
                                  name=f"nmr_bf{i}")
            nc.vector.tensor_scalar_mul(out=nmr_bf, in0=nmr, scalar1=16.0)
            return xt, diag_t, nmr_bf

        def emit_A_bias(i, nmr_bf):
            # emitted a beat after the stats so the PE's ptn transpose never
            # waits on the (2-ahead) DVE stats chain
            ptn = pst.tile([1, 128], BF16, tag="ptn", name=f"ptn{i}", bufs=1)
            nc.tensor.transpose(ptn, nmr_bf, ident_bf)
            nmr_row = a_small.tile([1, 128], BF16, tag="nmr_row",
                                   name=f"nmr_row{i}")
            nc.vector.tensor_copy(out=nmr_row, in_=ptn)
            # partition-broadcast via a rank-1 PE matmul (nothing else can
            # replicate a row across partitions) into a pt-ring slot
            pbc = pst.tile([128, 4, 128], F32, tag="pt", name=f"pbc{i}")
            nc.tensor.matmul(pbc[:, 0, :], ones_row, nmr_row,
                             start=True, stop=True)
            nc.vector.tensor_copy(out=xnT[:, KT + 1, i * 128:(i + 1) * 128],
                                  in_=pbc[:, 0, :])

        def emit_A_body(i, xt, diag_t, nmr_bf):
            emit_A_bias(i, nmr_bf)
            # 4 normalize-transposes per psum tile -> one wide plain-copy
            # evac (modulation lives in the host-folded weights)
            for k4 in range(KT // 4):
                pt = pst.tile([128, 4, 128], F32, tag="pt")
                for j in range(4):
                    k = k4 * 4 + j
                    # j=0's start zeroes the whole 2KB PSUM zero-region; a
                    # start on later j would re-mark earlier slices pending-
                    # zero for the region-spanning group
                    nc.tensor.matmul(pt[:, j, :], xt[:, k * 128:(k + 1) * 128],
                                     diag_t, start=(j == 0), stop=(j == 3),
                                     skip_group_check=True)
                dst = xnT[:, k4 * 4:k4 * 4 + 4, i * 128:(i + 1) * 128]
                nc.scalar.copy(out=dst, in_=pt)

        def emit_Bm_qk(pi, nat, m, evac_engines):
            # last DoubleRow pair is the two ones-tiles x replicated badd/256
            # rows, adding the modulation/bias term
            wt = wres_t[pi]
            for n in range(2):
                ps = psb.tile([128, 512], F32, tag="ps")
                for k in range(0, KTB, 2):
                    nc.tensor.matmul(ps, xnT[:, k:k + 2, m * 128:(m + 1) * 128],
                                     wt[:, k:k + 2, n * 512:(n + 1) * 512],
                                     start=(k == 0), stop=(k == KTB - 2),
                                     perf_mode=DR)
                dst = nat[:, m, n * 512:(n + 1) * 512]
                if evac_engines[n] == "act":
                    nc.scalar.activation(out=dst, in_=ps, func=AF.Identity,
                                         scale=1.0 / WS)
                else:
                    nc.vector.tensor_scalar_mul(out=dst, in0=ps,
                                                scalar1=1.0 / WS)

        def emit_Bm_v(m):
            # v = xn @ (wv_hi + wv_lo), both chains in one PSUM group; evac
            # to bf16 vtmp (ACT), then fp8 hi (ACT); the lo split is deferred
            # to the D/E boundary where DVE has slack.
            for n in range(2):
                ps = psb.tile([128, 512], F32, tag="ps")
                for k in range(0, KT, 2):
                    nc.tensor.matmul(ps, xnT[:, k:k + 2, m * 128:(m + 1) * 128],
                                     wres_t[2][:, k:k + 2, n * 512:(n + 1) * 512],
                                     start=(k == 0), stop=False, perf_mode=DR)
                    nc.tensor.matmul(ps, xnT[:, k:k + 2, m * 128:(m + 1) * 128],
                                     wres_t[3][:, k:k + 2, n * 512:(n + 1) * 512],
                                     start=False, stop=False, perf_mode=DR)
                # bias pair for the -mean*rstd*colsum(wv) term (badd_v itself
                # folds into vb via sum(softmax)=1)
                nc.tensor.matmul(ps, xnT[:, KT:KT + 2, m * 128:(m + 1) * 128],
                                 wres_t[2][:, KT:KT + 2, n * 512:(n + 1) * 512],
                                 start=False, stop=True, perf_mode=DR)
                sl = slice(n * 512, (n + 1) * 512)
                nc.scalar.activation(out=vtmp[:, m, sl], in_=ps,
                                     func=AF.Identity, scale=1.0 / WS)
            nc.scalar.copy(out=v2_hi[:, m, :], in_=vtmp[:, m, :])

        def emit_Cq_stats(m):
            # emitted one m ahead of the rope half so the ACT sqrt never
            # blocks the next B evacuations queued behind it
            qm = qnat[:, m, :]
            sq = c_p.tile([128, GCOLS], BF16, tag="sqk")
            # DVE 2x makes the square cheap here; Pool already carries the
            # q-rope half and would over-subscribe
            nc.vector.tensor_mul(out=sq, in0=qm, in1=qm)
            ssq = c_small.tile([128, HG], BF16, tag="ssq")
            with nc.allow_low_precision(reason="bf16 rms stats; ~0.3% scale"):
                nc.vector.reduce_sum(out=ssq,
                                     in_=sq.rearrange("p (h d) -> p h d", h=HG),
                                     axis=mybir.AxisListType.X)
            rrq = c_small.tile([128, HG], BF16, tag="rrq")
            nc.scalar.activation(out=rrq, in_=ssq, func=AF.Sqrt,
                                 bias=eps_t, scale=1.0 / HD)
            with nc.allow_low_precision(reason="bf16 1/rms scale; 0.4% rel"):
                nc.vector.reciprocal(out=rrq, in_=rrq)
            return rrq

        def emit_Cq_rope(m, rrq):
            qm = qnat[:, m, :]
            qmh = qm.rearrange("p (h c) -> p h c", h=HG)
            (ct, st) = cs_tiles[m]
            cb = ct.unsqueeze(1).broadcast_to([128, HG, RD // 2])
            sb_ = st.unsqueeze(1).broadcast_to([128, HG, RD // 2])
            if has_norm_w:
                nc.vector.tensor_mul(out=qmh, in0=qmh, in1=wqn_b)
            _emit_rope(qmh, cb, sb_, pool_half=True)
            rrq_b = rrq.unsqueeze(2).broadcast_to([128, HG, HD])
            nc.vector.tensor_mul(out=qmh, in0=qmh, in1=rrq_b)

        def emit_Ck_stats(m):
            km = knat[:, m, :]
            sk_ = c_p.tile([128, GCOLS], BF16, tag="sqk")
            # square on the idle gpsimd engine: DVE is the pacer here
            nc.gpsimd.tensor_mul(out=sk_, in0=km, in1=km)
            ssk = c_small.tile([128, HG], BF16, tag="ssk")
            with nc.allow_low_precision(reason="bf16 rms stats; ~0.3% scale"):
                nc.vector.reduce_sum(out=ssk,
                                     in_=sk_.rearrange("p (h d) -> p h d", h=HG),
                                     axis=mybir.AxisListType.X)
            nc.scalar.activation(out=rrk_all[:, m, :], in_=ssk, func=AF.Sqrt,
                                 bias=eps128_t, scale=1.0)
            nc.vector.reciprocal(out=rrk_all[:, m, :], in_=rrk_all[:, m, :])

        def emit_Ck_rope(m):
            km = knat[:, m, :]
            kmh = km.rearrange("p (h c) -> p h c", h=HG)
            (ct, st) = cs_tiles[m]
            cb = ct.unsqueeze(1).broadcast_to([128, HG, RD // 2])
            sb_ = st.unsqueeze(1).broadcast_to([128, HG, RD // 2])
            if has_norm_w:
                nc.vector.tensor_mul(out=kmh, in0=kmh, in1=wkn_b)
            _emit_rope(kmh, cb, sb_, pool_half=False)

        def _emit_rope(mh, cb, sb_, pool_half):
            # host-de-interleaved: cols [0:32]=x0, [32:64]=x1 per head.
            # pool_half routes the y1 branch to gpsimd (free in the q region).
            x0 = mh[:, :, 0:RD // 2]
            x1 = mh[:, :, RD // 2:RD]
            t0 = c_small.tile([128, HG, RD // 2], BF16, tag="t0")
            t1 = c_small.tile([128, HG, RD // 2], BF16, tag="t1")
            t2 = c_small.tile([128, HG, RD // 2], BF16, tag="t2")
            t3 = c_small.tile([128, HG, RD // 2], BF16, tag="t3")
            e = nc.gpsimd if pool_half else nc.vector
            nc.vector.tensor_mul(out=t0, in0=x0, in1=cb)
            nc.vector.tensor_mul(out=t1, in0=x1, in1=sb_)
            e.tensor_mul(out=t2, in0=x0, in1=sb_)
            e.tensor_mul(out=t3, in0=x1, in1=cb)
            nc.vector.tensor_sub(out=x0, in0=t0, in1=t1)
            e.tensor_add(out=x1, in0=t2, in1=t3)

        # ---- fused A + B-q emission ----
        xt01 = {i: emit_A_dma(i) for i in (0, 1)}
        init_misc()
        pend = {i: emit_A_stats(i, xt01.pop(i)) for i in (0, 1)}
        for i in range(NT):
            if i + 2 < NT:
                pend[i + 2] = emit_A_stats(i + 2)
            emit_A_body(i, *pend.pop(i))
            # all wq chunks MUST be emitted before the first emit_Bm below
            # (deps only order reads after already-emitted writes)
            if i == 0:
                load_wres(0, range(0, 2))
            elif i == 1:
                load_wres(0, range(2, 4))
            elif i == 3:
                load_wres(1)          # wk, consumed after A+B-q
            if i >= 1:
                emit_Bm_qk(0, qnat, i - 1, ("act", "act"))
        emit_Bm_qk(0, qnat, NT - 1, ("act", "act"))

        # deferred misc loads (consumed in phases C/E/F) — single batched DMAs
        # so their dispatch overhead doesn't contend with the x/w streams
        nc.sync.dma_start(out=gate_sb, in_=gate)
        nc.sync.dma_start(out=vb_sb, in_=vb)
        if has_norm_w:
            def _bcast_from_dram(ap, parts, reps=None):
                newap = [[0, parts]]
                if reps is not None:
                    newap.append([0, reps])
                newap += list(ap.ap)
                return bass.AP(tensor=ap.tensor, offset=ap.offset, ap=newap)
            nc.sync.dma_start(out=wqn_b, in_=_bcast_from_dram(wqn, 128, reps=HG))
            nc.sync.dma_start(out=wkn_b, in_=_bcast_from_dram(wkn, 128, reps=HG))
        nc.sync.dma_start(out=cos_all, in_=cos.rearrange("(m p) f -> p m f", p=128))
        nc.sync.dma_start(out=sin_all, in_=sin.rearrange("(m p) f -> p m f", p=128))

        # ---- B-k (PE) interleaved with C-q (DVE/Pool) per m-tile ----
        rrq_pend = {0: emit_Cq_stats(0)}
        for m in range(NT):
            emit_Bm_qk(1, knat, m, ("act", "act"))
            if m + 1 < NT:
                rrq_pend[m + 1] = emit_Cq_stats(m + 1)
            emit_Cq_rope(m, rrq_pend.pop(m))
            if m == 0:
                load_wres(2)          # wv_hi
                load_wres(3)          # wv_lo
        # ---- B-v (PE) interleaved with C-k (DVE/Pool) ----
        emit_Ck_stats(0)
        for m in range(NT):
            emit_Bm_v(m)
            if m + 1 < NT:
                emit_Ck_stats(m + 1)
            emit_Ck_rope(m)

        psb_cm.__exit__(None, None, None)
        pst_cm.__exit__(None, None, None)
        c_small_cm.__exit__(None, None, None)
        c_cm.__exit__(None, None, None)
        a_small_cm.__exit__(None, None, None)
        a_cm.__exit__(None, None, None)
        xnT_cm.__exit__(None, None, None)
        wres_cm.__exit__(None, None, None)

        # ======== phases D/E/F share the left stack ================================
        oT_cm = tc.tile_pool(name="oT", bufs=1, side="left")
        oT_p = oT_cm.__enter__()
        oT_hi = oT_p.tile([128, HG, S], F8)
        oT_lo = oT_p.tile([128, HG, S], F8)

        # wo prefetch (resident; consumed in phase F)
        wo_cm = tc.tile_pool(name="wo_res", bufs=1, side="left")
        wo_p = wo_cm.__enter__()
        wo_hi_sb = wo_p.tile([128, KT, HG, 128], F8)
        wo_lo_sb = wo_p.tile([128, KT, HG, 128], F8)

        # ---- phase D: transpose q, k -> [hd, s] per head (batched evacs)
        # q evacs on ACT (DVE is finishing C-k); k evacs on DVE; the v lo
        # split (SBUF-only, deferred from B-v) fills DVE alongside.
        qkT_cm = tc.tile_pool(name="qkT", bufs=1, side="left")
        qkT_p = qkT_cm.__enter__()
        qT = qkT_p.tile([128, HG, S], BF16)
        kT = qkT_p.tile([128, HG, S], BF16)
        pst2_cm = tc.tile_pool(name="ps_tr2", bufs=4, space="PSUM")
        pst2 = pst2_cm.__enter__()

        def emit_D(nat, dstT, h, evac):
            pt8 = pst2.tile([128, S], BF16, tag="pt8")
            for m in range(NT):
                nc.tensor.transpose(pt8[:, m * 128:(m + 1) * 128],
                                    nat[:, m, h * 128:(h + 1) * 128],
                                    ident_bf)
            dst = dstT[:, h, :]
            if evac == "act":
                nc.scalar.copy(out=dst, in_=pt8)
            else:
                nc.vector.tensor_copy(out=dst, in_=pt8)

        for h in range(HG):
            emit_D(qnat, qT, h, "act" if h % 2 == 0 else "dve")
            if h % 2 == 0:
                m = h // 2 * 2
                with nc.allow_low_precision(reason="fp8 v lo split"):
                    nc.vector.tensor_sub(out=v2_lo[:, m, :],
                                         in0=vtmp[:, m, :], in1=v2_hi[:, m, :])
                    nc.vector.tensor_sub(out=v2_lo[:, m + 1, :],
                                         in0=vtmp[:, m + 1, :],
                                         in1=v2_hi[:, m + 1, :])
        for h in range(HG):
            emit_D(knat, kT, h, "act" if h % 2 == 1 else "dve")
        pst2_cm.__exit__(None, None, None)
        # (natqk stays open through E/F: closing it between D and E would
        # add a pool barrier; SBUF has room)

        # wo loads (consumed in F; DMA-engine time hidden under phase E)
        for m in range(KT):
            nc.sync.dma_start(out=wo_hi_sb[:, m, :, :], in_=wo[:, m, 0, :, :])
            nc.sync.dma_start(out=wo_lo_sb[:, m, :, :], in_=wo[:, m, 1, :, :])

        # ---- phase E: attention per head (fp8 DoubleRow attn*v) ----
        at2_cm = tc.tile_pool(name="at2", bufs=2, side="left")
        at2_p = at2_cm.__enter__()
        rs_cm = tc.tile_pool(name="rsb", bufs=2, side="left")
        rs_p = rs_cm.__enter__()
        pssc_cm = tc.tile_pool(name="ps_sc", bufs=2, space="PSUM")
        pssc = pssc_cm.__enter__()
        pso_cm = tc.tile_pool(name="ps_o", bufs=1, space="PSUM")
        pso = pso_cm.__enter__()
        psdn_cm = tc.tile_pool(name="ps_dn", bufs=2, space="PSUM")
        psdn = psdn_cm.__enter__()

        def emit_scores(h, m):
            sc = pssc.tile([128, S], F32, tag="sc")
            lhs_k = kT[:, h, m * 128:(m + 1) * 128]
            nc.tensor.matmul(sc[:, 0:512], lhs_k, qT[:, h, 0:512],
                             start=True, stop=True)
            nc.tensor.matmul(sc[:, 512:1024], lhs_k, qT[:, h, 512:1024],
                             start=True, stop=True)
            return sc

        for h in range(HG):
            o_ps = pso.tile([128, S], F32, tag="o_ps", name=f"o_ps{h}")
            dns = [psdn.tile([128, 512], F32, tag="dn", name=f"dn{h}_{g}")
                   for g in range(2)]
            sc_next = emit_scores(h, 0)
            at = None
            for m in range(NT):
                sc = sc_next
                sc_next = emit_scores(h, m + 1) if m + 1 < NT else None
                if m % 2 == 0:
                    at = at2_p.tile([128, 2, S], F8, tag="at", name="at")
                # fp8 attn weights: exp(score*rrk - C) stays under e4m3 240;
                # the e^-C factor cancels between numerator and denominator
                nc.scalar.activation(out=at[:, m % 2, :], in_=sc, func=AF.Exp,
                                     scale=rrk_all[:, m, h:h + 1], bias=negc_t)
                if m % 2 == 1:
                    mp = m // 2
                    first, last = (mp == 0), (mp == NT // 2 - 1)
                    for g in range(2):
                        # denominator on PE: ones^T @ at, replicated rows
                        nc.tensor.matmul(dns[g], ones8,
                                         at[:, :, g * 512:(g + 1) * 512],
                                         start=first, stop=last, perf_mode=DR)
                    for vi, vsrc in enumerate((v2_hi, v2_lo)):
                        v_mh = vsrc[:, 2 * mp:2 * mp + 2, h * 128:(h + 1) * 128]
                        nc.tensor.matmul(o_ps[:, 0:512], v_mh, at[:, :, 0:512],
                                         start=(first and vi == 0),
                                         stop=(last and vi == 1), perf_mode=DR)
                        nc.tensor.matmul(o_ps[:, 512:1024], v_mh,
                                         at[:, :, 512:1024],
                                         start=(first and vi == 0),
                                         stop=(last and vi == 1), perf_mode=DR)
            for g in range(2):
                sl = slice(g * 512, (g + 1) * 512)
                sums = rs_p.tile([128, 512], F32, tag="sums", name=f"sums{h}{g}")
                nc.vector.reciprocal(out=sums, in_=dns[g])
                obf = rs_p.tile([128, 512], BF16, tag="obf", name=f"obf{h}{g}")
                nc.vector.tensor_mul(out=obf, in0=o_ps[:, sl], in1=sums)
                nc.vector.tensor_copy(out=oT_hi[:, h, sl], in_=obf)
                with nc.allow_low_precision(reason="fp8 o lo split"):
                    nc.vector.tensor_sub(out=oT_lo[:, h, sl], in0=obf,
                                         in1=oT_hi[:, h, sl])

        # ---- phase F: out projection as three fp8 DoubleRow chains
        # (hi*hi + hi*lo + lo*hi; the dropped lo*lo term is ~0.1%^2), emitted
        # inside E's psum pools so no pool barrier splits E/F
        F_CHAINS = 3
        F_KP = HG // 2

        def emit_F_mm(po, m, ci, kp):
            wsrc, osrc = ((wo_hi_sb, oT_hi), (wo_hi_sb, oT_lo),
                          (wo_lo_sb, oT_hi))[ci]
            first = (ci == 0 and kp == 0)
            last = (ci == F_CHAINS - 1 and kp == F_KP - 1)
            lhs = wsrc[:, m, 2 * kp:2 * kp + 2, :]
            nc.tensor.matmul(po[:, 0:512], lhs, osrc[:, 2 * kp:2 * kp + 2, 0:512],
                             start=first, stop=last, perf_mode=DR)
            nc.tensor.matmul(po[:, 512:1024], lhs,
                             osrc[:, 2 * kp:2 * kp + 2, 512:1024],
                             start=first, stop=last, perf_mode=DR)

        def emit_F_evac(po, m):
            ot_t = rs_p.tile([128, S], BF16, tag="ot_t", name=f"ot_t{m}")
            if m == KT - 1:
                # the kernel's tail: halve the evac+store chain so the first
                # half's DMA overlaps the second half's evacuation
                for g in (0, 1):
                    sl = slice(g * 512, (g + 1) * 512)
                    nc.vector.tensor_scalar(out=ot_t[:, sl], in0=po[:, sl],
                                            scalar1=gate_sb[:, m:m + 1],
                                            scalar2=vb_sb[:, m:m + 1],
                                            op0=ALU.mult, op1=ALU.add)
                    nc.sync.dma_start(out=out_t[m * 128:(m + 1) * 128, sl],
                                      in_=ot_t[:, sl])
                return
            if m % 2 == 0:
                nc.scalar.activation(out=ot_t, in_=po, func=AF.Identity,
                                     bias=vb_sb[:, m:m + 1],
                                     scale=gate_sb[:, m:m + 1])
            else:
                nc.vector.tensor_scalar(out=ot_t, in0=po,
                                        scalar1=gate_sb[:, m:m + 1],
                                        scalar2=vb_sb[:, m:m + 1],
                                        op0=ALU.mult, op1=ALU.add)
            nc.sync.dma_start(out=out_t[m * 128:(m + 1) * 128, :], in_=ot_t)

        def alloc_po(m):
            # alternate between the two dead attention psum pools for slack
            return (pssc.tile([128, S], F32, tag="sc", name=f"po{m}")
                    if m % 2 == 0
                    else pso.tile([128, S], F32, tag="o_ps", name=f"po{m}"))

        # first two output tiles interleave their chain loops so F has dense
        # matmuls queued before it needs the last head's normalization
        steps = [(ci, kp) for ci in range(F_CHAINS) for kp in range(F_KP)]
        po01 = {m: alloc_po(m) for m in (0, 1)}
        for m in (0, 1):
            for (ci, kp) in steps[:-2]:
                emit_F_mm(po01[m], m, ci, kp)
        for m in (0, 1):
            for (ci, kp) in steps[-2:]:
                emit_F_mm(po01[m], m, ci, kp)
            emit_F_evac(po01[m], m)
        for m in range(2, KT):
            po = alloc_po(m)
            for (ci, kp) in steps:
                emit_F_mm(po, m, ci, kp)
            emit_F_evac(po, m)

        psdn_cm.__exit__(None, None, None)
        pso_cm.__exit__(None, None, None)
        pssc_cm.__exit__(None, None, None)
        rs_cm.__exit__(None, None, None)
        at2_cm.__exit__(None, None, None)
        qkT_cm.__exit__(None, None, None)
        natqk_cm.__exit__(None, None, None)
        v_cm.__exit__(None, None, None)
        wo_cm.__exit__(None, None, None)
        oT_cm.__exit__(None, None, None)
        misc_cm.__exit__(None, None, None)

    nc.compile()
    return nc


_NC_CACHE = {}


def _get_nc(has_norm_w):
    if has_norm_w not in _NC_CACHE:
        _NC_CACHE[has_norm_w] = build_nc(has_norm_w)
    return _NC_CACHE[has_norm_w]


# per-head column permutation de-interleaving rope pairs:
# [0,2,..,62, 1,3,..,63, 64..127] within each head's 128 columns
def _qk_perm():
    base = np.concatenate([np.arange(0, RD, 2), np.arange(1, RD, 2),
                           np.arange(RD, HD)])
    return np.concatenate([h * HD + base for h in range(HG)])


_PERM = _qk_perm()
F8NP = ml_dtypes.float8_e4m3


def prep_in_maps(x, mod, cos, sin, qkv_w, qkv_b, mod_w, mod_b, out_w, out_b,
                 norm_q_w, norm_k_w):
    """Host-side sharding. Returns (in_maps, flags, x_np)."""
    x = np.asarray(x, dtype=np.float32)
    m3 = np.asarray(mod, np.float32) @ np.asarray(mod_w, np.float32) \
        + np.asarray(mod_b, np.float32)
    bias, scale, gatef = np.split(m3, 3, axis=-1)          # [B, D] each
    scale1p = (1.0 + scale).astype(np.float32)

    qkv_b = np.asarray(qkv_b, np.float32)
    has_norm_w = not (np.allclose(norm_q_w, 1.0) and np.allclose(norm_k_w, 1.0))

    cosb = np.ascontiguousarray(np.asarray(cos, np.float32).astype(ml_dtypes.bfloat16))
    sinb = np.ascontiguousarray(np.asarray(sin, np.float32).astype(ml_dtypes.bfloat16))
    qkv_w = np.asarray(qkv_w, np.float32)
    out_w = np.asarray(out_w, np.float32)
    out_b = np.asarray(out_b, np.float32)

    # per-head rope-pair de-interleave permutation (shared by q and k; inner
    # products and rms are invariant; cos/sin indices line up with x0/x1 halves)
    perm = _PERM
    hd_perm = np.concatenate([np.arange(0, RD, 2), np.arange(1, RD, 2),
                              np.arange(RD, HD)])

    def with_bias_tile(w_scaled_f8, badd, colsum_scaled):
        # tile16 rows (x ones): badd*WS/128; tile17 rows (x 16*nmr):
        # colsum(W_fp8)*WS / (128*16) — the rank-1 mean-subtraction term
        brow = np.broadcast_to((badd * (WS / 128.0))[None, :],
                               (128, badd.shape[0]))
        crow = np.broadcast_to((colsum_scaled / 2048.0)[None, :],
                               (128, badd.shape[0]))
        return np.ascontiguousarray(np.concatenate(
            [w_scaled_f8, brow.astype(F8NP), crow.astype(F8NP)], axis=0))

    in_maps = []
    for c in range(N_CORES):
        b, g = divmod(c, 2)
        lo = g * GCOLS
        s1p = scale1p[b][:, None]                          # [D, 1]
        wq_c = (qkv_w[:, lo:lo + GCOLS] * s1p)[:, perm]
        wk_c = (qkv_w[:, 2048 + lo:2048 + lo + GCOLS] * s1p)[:, perm]
        wv_c = qkv_w[:, 4096 + lo:4096 + lo + GCOLS] * s1p
        # modulation/qkv bias rows: b @ W (+ qkv bias), per projection
        badd_q = bias[b] @ qkv_w[:, lo:lo + GCOLS] + qkv_b[lo:lo + GCOLS]
        badd_k = bias[b] @ qkv_w[:, 2048 + lo:2048 + lo + GCOLS] \
            + qkv_b[2048 + lo:2048 + lo + GCOLS]
        badd_v = bias[b] @ qkv_w[:, 4096 + lo:4096 + lo + GCOLS] \
            + qkv_b[4096 + lo:4096 + lo + GCOLS]
        wo_c = out_w[lo:lo + GCOLS, :]                      # [GCOLS, D]
        # wo_l[p, m, kb, c] = wo_c[kb*128+p, m*128+c], fp8 hi/lo pair
        wo_l = np.transpose(
            wo_c.reshape(HG, 128, KT, 128), (1, 2, 0, 3)) * WS2
        wo_hi = wo_l.astype(F8NP)
        wo_lo = (wo_l - wo_hi.astype(np.float32)).astype(F8NP)
        wo_pair = np.stack([wo_hi, wo_lo], axis=2)   # [128, KT, 2, HG, 128]
        # v weights as an fp8 hi+lo pair at a shared scale: lo lands in the
        # e4m3 subnormal range, leaving ~0.1% net weight error
        wvh_c = (wv_c * WS).astype(F8NP)
        wvl_c = (wv_c * WS - wvh_c.astype(np.float32)).astype(F8NP)
        zrow = np.zeros((256, GCOLS), np.float32)
        wq_f8 = (wq_c * WS).astype(F8NP)
        wk_f8 = (wk_c * WS).astype(F8NP)
        # v bias folds into the output bias: sum(softmax)=1 makes o shift by
        # badd_v exactly, i.e. the final out shifts by (badd_v @ wo) * gate
        vb_full = badd_v @ wo_c                             # [D]
        if g == 0:
            vb_full = vb_full + out_b
        vbf = (vb_full * gatef[b]).astype(np.float32)
        im = {
            "x": np.ascontiguousarray(x[b].astype(ml_dtypes.bfloat16)),
            "cos": cosb, "sin": sinb,
            "wq": with_bias_tile(wq_f8, badd_q[perm],
                                 wq_f8.astype(np.float32).sum(0)),
            "wk": with_bias_tile(wk_f8, badd_k[perm],
                                 wk_f8.astype(np.float32).sum(0)),
            "wvh": with_bias_tile(wvh_c, np.zeros(GCOLS, np.float32),
                                  wvh_c.astype(np.float32).sum(0)
                                  + wvl_c.astype(np.float32).sum(0)),
            "wvl": np.ascontiguousarray(
                np.concatenate([wvl_c, zrow.astype(F8NP)], axis=0)),
            "wo": np.ascontiguousarray(wo_pair),
            "gate": np.ascontiguousarray(
                (gatef[b] / WS2).reshape(KT, 128).T),
            "vb": np.ascontiguousarray(vbf.reshape(KT, 128).T),
        }
        if has_norm_w:
            im["wqn"] = np.ascontiguousarray(
                np.asarray(norm_q_w, np.float32)[hd_perm].astype(ml_dtypes.bfloat16))
            im["wkn"] = np.ascontiguousarray(
                np.asarray(norm_k_w, np.float32)[hd_perm].astype(ml_dtypes.bfloat16))
        in_maps.append(im)
    return in_maps, has_norm_w, x


def gather(results, x):
    B = x.shape[0]
    outs = []
    for b in range(B):
        p = results[2 * b]["out_t"].astype(np.float32) \
            + results[2 * b + 1]["out_t"].astype(np.float32)        # [D, S]
        outs.append(p.T + x[b])
    return np.stack(outs).astype(np.float32)


def kernel(**inputs) -> np.ndarray:
    in_maps, has_norm_w, x = prep_in_maps(**inputs)
    nc = _get_nc(has_norm_w)
    res = run_bass_kernel_spmd(nc, in_maps, core_ids=list(range(N_CORES)))
    return gather(res.results, x)


if __name__ == "__main__":
    import time
    t0 = time.time()
    nc = build_nc(False)
    print("build+compile ok in", time.time() - t0, "s")


# revision 80
# speedup vs baseline: 1.1989x; 1.0110x over previous
"""Trainium2 Bass kernel for ModalityAttention (B=4, S=1024, D=2048, H=16, HD=128, RD=64).

Sharding: 8 cores = 4 batches x 2 head-groups (8 heads each).
Each core computes, for its (batch b, head-group g):
  layernorm(x[b]) -> qkv projection (modulation host-folded into the
  weights) for its 8 heads -> rmsnorm + rope -> attention
  -> partial out-projection (transposed layout) with gate folded in.
Host gathers: out[b] = (partial_g0 + partial_g1).T + x[b]

Modulation fold: qkv(xmod) with xmod=(1+s)*xn+b equals
xn @ ((1+s)[d] * W) + b @ W.  The row-scale goes into the host fp8
weight cast (per-core batch); the additive row b@W rides the matmul as
a 17th contraction tile (fp8 ones in xnT x replicated badd/128 rows)
for q/k, and for v folds into the output bias via sum(softmax)=1:
adding c[col] to v shifts o by exactly c, i.e. out by (c @ wo) -- a
host-side vb adjustment.  qkv_b folds into the same terms.

Precision strategy (fp8 e4m3 DoubleRow matmuls where softmax/rms
renormalization absorbs the quantization noise, near-exact elsewhere):
  - q,k projections: fp8 DoubleRow (xn fp8, weights fp8*256).
  - v projection: fp8 DoubleRow with the weight split into hi+lo fp8
    at a shared scale, accumulating in one PSUM group -> ~0.1% weight
    error (the v path feeds the output linearly, it cannot absorb noise).
  - attn*v: fp8 DoubleRow with v split hi+lo on device; exp emits fp8
    with a constant -C_EXP bias (cancels in the softmax ratio) to stay
    under the e4m3 +-240 overflow.
  - softmax denominator: ones-vector DoubleRow matmul on PE whose
    replicated output rows make the broadcast free.
  - scores and out projection: bf16.

Engine balance: gpsimd (Pool) takes SBUF-only elementwise work (k
squares, part of q rope) since it cannot touch PSUM; evacs batch 4
transposes per PSUM tile to amortize the fp8 no-2x penalty on DVE/ACT.
"""
import os, sys

for _p in ("/opt/trn_rl_repo", "/root/.axon_site/_ro/trn_rl_repo", "/root/.axon_site"):
    if os.path.isdir(_p) and _p not in sys.path:
        sys.path.insert(0, _p)

import numpy as np
import ml_dtypes
import concourse.bass as bass
import concourse.bacc as bacc
import concourse.mybir as mybir
import concourse.tile as tile
from concourse import bass_isa
from concourse.masks import make_identity
from concourse.bass_utils import run_bass_kernel_spmd

F32 = mybir.dt.float32
BF16 = mybir.dt.bfloat16
F8 = mybir.dt.float8e4
DR = mybir.MatmulPerfMode.DoubleRow
AF = mybir.ActivationFunctionType
ALU = mybir.AluOpType
WS = 256.0      # host-side qkv weight scale (fp8 e4m3 subnormal floor)
WS2 = 256.0     # host-side out-proj weight scale (descale rides the gate)
C_EXP = 2.0     # exp(score - C_EXP): keeps fp8 attn weights under 240
S, D, HG, HD, RD = 1024, 2048, 8, 128, 64
NT = S // 128        # 8 s-tiles
KT = D // 128        # 16 d-tiles
KTB = KT + 2         # + two ones/badd bias tiles for q,k (a DoubleRow pair)
GCOLS = HG * HD      # 1024 columns per group per projection
EPS = 1e-6
N_CORES = 8


def build_nc(has_norm_w: bool):
    nc = bacc.Bacc("TRN2", target_bir_lowering=False, debug=False,
                   enable_asserts=True, num_devices=N_CORES)

    x = nc.dram_tensor("x", [S, D], BF16, kind="ExternalInput").ap()
    cos = nc.dram_tensor("cos", [S, RD // 2], BF16, kind="ExternalInput").ap()
    sin = nc.dram_tensor("sin", [S, RD // 2], BF16, kind="ExternalInput").ap()
    wq = nc.dram_tensor("wq", [KTB * 128, GCOLS], F8, kind="ExternalInput").ap()
    wk = nc.dram_tensor("wk", [KTB * 128, GCOLS], F8, kind="ExternalInput").ap()
    wvh = nc.dram_tensor("wvh", [KTB * 128, GCOLS], F8, kind="ExternalInput").ap()
    wvl = nc.dram_tensor("wvl", [KTB * 128, GCOLS], F8, kind="ExternalInput").ap()
    # wo pre-laid-out on host as an fp8 hi/lo pair:
    # wo[p, m, hl, kb, c] = hi/lo split of wo[kb*128+p, m*128+c] * WS2
    wo = nc.dram_tensor("wo", [128, KT, 2, HG, 128], F8,
                        kind="ExternalInput").ap()
    gate = nc.dram_tensor("gate", [128, KT], F32, kind="ExternalInput").ap()
    vb = nc.dram_tensor("vb", [128, KT], F32, kind="ExternalInput").ap()
    if has_norm_w:
        wqn = nc.dram_tensor("wqn", [HD], BF16, kind="ExternalInput").ap()
        wkn = nc.dram_tensor("wkn", [HD], BF16, kind="ExternalInput").ap()
    # bf16 output halves the store DMA; the host gather sums the two
    # head-group partials in f32 (bf16 partial rounding ~1e-4 of the answer)
    out_t = nc.dram_tensor("out_t", [D, S], BF16, kind="ExternalOutput").ap()

    with tile.TileContext(nc) as tc:
        # ======== LEFT stack bottom: small persistent constants ====================
        misc_cm = tc.tile_pool(name="misc", bufs=1, side="left")
        misc = misc_cm.__enter__()
        ident_bf = misc.tile([128, 128], BF16)
        eps_t = misc.tile([128, 1], F32)
        eps128_t = misc.tile([128, 1], F32)
        # full-width fp8 ones: the DoubleRow denominator matmul then emits the
        # softmax sum replicated across all 128 partitions (same cost — matmul
        # time only scales with the moving free size), so no broadcast needed
        ones8 = misc.tile([128, 2, 128], F8)
        negc_t = misc.tile([128, 1], F32)
        ones_row = misc.tile([1, 128], BF16)

        def init_misc():
            # emitted after the first x DMAs so the load starts immediately;
            # memsets ride gpsimd to keep the DVE queue clear for LN stats
            make_identity(nc, ident_bf)
            nc.gpsimd.memset(eps_t, EPS)
            nc.gpsimd.memset(eps128_t, HD * EPS)
            nc.gpsimd.memset(ones8, 1.0)
            nc.gpsimd.memset(negc_t, -C_EXP)
            nc.gpsimd.memset(ones_row, 1.0)
        gate_sb = misc.tile([128, KT], F32)
        vb_sb = misc.tile([128, KT], F32)
        rrk_all = misc.tile([128, NT, HG], F32)   # scaled k-rms reciprocals
        if has_norm_w:
            wqn_b = misc.tile([128, HG, HD], BF16)
            wkn_b = misc.tile([128, HG, HD], BF16)
        cos_all = misc.tile([128, NT, RD // 2], BF16)
        sin_all = misc.tile([128, NT, RD // 2], BF16)
        cs_tiles = [(cos_all[:, m, :], sin_all[:, m, :]) for m in range(NT)]

        # ======== RIGHT stack: natural-layout q/k (bf16) and v hi/lo (fp8) =========
        v_cm = tc.tile_pool(name="vpool", bufs=1, side="right")
        v_p = v_cm.__enter__()
        v2_hi = v_p.tile([128, NT, GCOLS], F8)
        v2_lo = v_p.tile([128, NT, GCOLS], F8)
        vtmp = v_p.tile([128, NT, GCOLS], BF16)
        natqk_cm = tc.tile_pool(name="natqk", bufs=1, side="right")
        natqk = natqk_cm.__enter__()
        qnat = natqk.tile([128, NT, GCOLS], BF16)
        knat = natqk.tile([128, NT, GCOLS], BF16)

        # resident qkv weights, triple-buffered: wv_hi/wv_lo both arrive
        # during B-k (tile 3 reuses tile 0's space, freed after B-q)
        wres_cm = tc.tile_pool(name="wres", bufs=3, side="right")
        wres_p = wres_cm.__enter__()
        wres_t = [wres_p.tile([128, KTB, GCOLS], F8, tag="wres", name=f"wres{pi}")
                  for pi in range(4)]
        wq_r = wq.rearrange("(k p) c -> p k c", p=128)
        wk_r = wk.rearrange("(k p) c -> p k c", p=128)
        wvh_r = wvh.rearrange("(k p) c -> p k c", p=128)
        wvl_r = wvl.rearrange("(k p) c -> p k c", p=128)
        w_drams = (wq_r, wk_r, wvh_r, wvl_r)

        def load_wres(pi, chunks=range(4)):
            for c4 in chunks:
                lo = c4 * 4
                hi = KTB if c4 == 3 else lo + 4   # last chunk carries tile 16
                nc.sync.dma_start(out=wres_t[pi][:, lo:hi, :],
                                  in_=w_drams[pi][:, lo:hi, :])

        # ======== phase A pools ====================================================
        xnT_cm = tc.tile_pool(name="xnT", bufs=1, side="left")
        xnT_p = xnT_cm.__enter__()
        xnT = xnT_p.tile([128, KTB, S], F8)  # [d_in_tile, d_tile, s]
        # bias-tile pair: tile16 = ones (x badd row), tile17 = 16*nmr per
        # column (x colsum(W)/16 row) — the mean-subtraction term
        # (-mean*rstd*colsum) is rank-1 and rides the same DoubleRow pair
        nc.gpsimd.memset(xnT[:, KT, :], 1.0)

        a_cm = tc.tile_pool(name="phA", bufs=3, side="left")
        a_p = a_cm.__enter__()
        a_small_cm = tc.tile_pool(name="phA_small", bufs=6, side="left")
        a_small = a_small_cm.__enter__()
        pst_cm = tc.tile_pool(name="ps_tr", bufs=4, space="PSUM")
        pst = pst_cm.__enter__()
        # pt: 4 bufs x 1 bank; ptn: 1 buf -> 5 banks + ps_qkv's 3 = 8

        # phase C pools opened early so later emission can overlap B on DVE
        c_cm = tc.tile_pool(name="phC", bufs=2, side="left")
        c_p = c_cm.__enter__()
        c_small_cm = tc.tile_pool(name="phC_small", bufs=2, side="left")
        c_small = c_small_cm.__enter__()

        psb_cm = tc.tile_pool(name="ps_qkv", bufs=3, space="PSUM")
        psb = psb_cm.__enter__()

        # ---- emit helpers ----
        def emit_A_dma(i):
            xt = a_p.tile([128, D], BF16, tag="xt", name=f"xt{i}")
            nc.sync.dma_start(out=xt[:, 0:D // 2],
                              in_=x[i * 128:(i + 1) * 128, 0:D // 2])
            nc.sync.dma_start(out=xt[:, D // 2:D],
                              in_=x[i * 128:(i + 1) * 128, D // 2:D])
            return xt

        def emit_A_stats(i, xt=None):
            """x load + LN statistics; emitted 2 s-tiles ahead of the body so
            the serial stats chain never gates the PE transposes."""
            if xt is None:
                xt = emit_A_dma(i)
            stats = a_small.tile([128, 4, 6], F32, tag="stats",
                                 name=f"stats{i}")
            xv = xt.rearrange("p (c f) -> p c f", c=4)
            for c in range(4):
                nc.vector.bn_stats(out=stats[:, c, :], in_=xv[:, c, :])
            mv = a_small.tile([128, 2], F32, tag="mv", name=f"mv{i}")
            nc.vector.bn_aggr(out=mv, in_=stats)
            rstd = a_small.tile([128, 1], F32, tag="rstd", name=f"rstd{i}")
            nc.scalar.activation(out=rstd, in_=mv[:, 1:2],
                                 func=AF.Abs_reciprocal_sqrt, bias=eps_t,
                                 scale=1.0)
            # LN-apply rides the transpose matmul: rhs = diag(rstd); the
            # -mean*rstd term lands in xnT's 17th column tile (16*nmr in fp8,
            # contracted with host colsum(W)/16 rows by the bias DR pair)
            diag_t = a_small.tile([128, 128], BF16, tag="diag",
                                  name=f"diag{i}")
            nc.vector.tensor_scalar_mul(out=diag_t, in0=ident_bf,
                                        scalar1=rstd)
            nmr_bf = a_small.tile([128, 1], BF16, tag="nmr_bf",
                                  name=f"nmr_bf{i}")
            nc.vector.tensor_scalar(out=nmr_bf, in0=mv[:, 0:1], scalar1=rstd,
                                    scalar2=-16.0, op0=ALU.mult, op1=ALU.mult)
            return xt, diag_t, nmr_bf

        def emit_A_bias(i, nmr_bf):
            # emitted a beat after the stats so the PE's ptn transpose never
            # waits on the (2-ahead) DVE stats chain
            ptn = pst.tile([1, 128], BF16, tag="ptn", name=f"ptn{i}", bufs=1)
            nc.tensor.transpose(ptn, nmr_bf, ident_bf)
            nmr_row = a_small.tile([1, 128], BF16, tag="nmr_row",
                                   name=f"nmr_row{i}")
            nc.vector.tensor_copy(out=nmr_row, in_=ptn)
            # partition-broadcast via a rank-1 PE matmul (nothing else can
            # replicate a row across partitions) into a pt-ring slot
            pbc = pst.tile([128, 4, 128], F32, tag="pt", name=f"pbc{i}")
            nc.tensor.matmul(pbc[:, 0, :], ones_row, nmr_row,
                             start=True, stop=True)
            nc.vector.tensor_copy(out=xnT[:, KT + 1, i * 128:(i + 1) * 128],
                                  in_=pbc[:, 0, :])

        def emit_A_body(i, xt, diag_t, nmr_bf):
            emit_A_bias(i, nmr_bf)
            # 4 normalize-transposes per psum tile -> one wide plain-copy
            # evac (modulation lives in the host-folded weights)
            for k4 in range(KT // 4):
                pt = pst.tile([128, 4, 128], F32, tag="pt")
                for j in range(4):
                    k = k4 * 4 + j
                    # j=0's start zeroes the whole 2KB PSUM zero-region; a
                    # start on later j would re-mark earlier slices pending-
                    # zero for the region-spanning group
                    nc.tensor.matmul(pt[:, j, :], xt[:, k * 128:(k + 1) * 128],
                                     diag_t, start=(j == 0), stop=(j == 3),
                                     skip_group_check=True)
                dst = xnT[:, k4 * 4:k4 * 4 + 4, i * 128:(i + 1) * 128]
                nc.scalar.copy(out=dst, in_=pt)

        def emit_Bm_qk(pi, nat, m, evac_engines):
            # last DoubleRow pair is the two ones-tiles x replicated badd/256
            # rows, adding the modulation/bias term
            wt = wres_t[pi]
            for n in range(2):
                ps = psb.tile([128, 512], F32, tag="ps")
                for k in range(0, KTB, 2):
                    nc.tensor.matmul(ps, xnT[:, k:k + 2, m * 128:(m + 1) * 128],
                                     wt[:, k:k + 2, n * 512:(n + 1) * 512],
                                     start=(k == 0), stop=(k == KTB - 2),
                                     perf_mode=DR)
                dst = nat[:, m, n * 512:(n + 1) * 512]
                if evac_engines[n] == "act":
                    nc.scalar.activation(out=dst, in_=ps, func=AF.Identity,
                                         scale=1.0 / WS)
                else:
                    nc.vector.tensor_scalar_mul(out=dst, in0=ps,
                                                scalar1=1.0 / WS)

        def emit_Bm_v(m):
            # v = xn @ (wv_hi + wv_lo), both chains in one PSUM group; evac
            # to bf16 vtmp (ACT), then fp8 hi (ACT); the lo split is deferred
            # to the D/E boundary where DVE has slack.
            for n in range(2):
                ps = psb.tile([128, 512], F32, tag="ps")
                for k in range(0, KT, 2):
                    nc.tensor.matmul(ps, xnT[:, k:k + 2, m * 128:(m + 1) * 128],
                                     wres_t[2][:, k:k + 2, n * 512:(n + 1) * 512],
                                     start=(k == 0), stop=False, perf_mode=DR)
                    nc.tensor.matmul(ps, xnT[:, k:k + 2, m * 128:(m + 1) * 128],
                                     wres_t[3][:, k:k + 2, n * 512:(n + 1) * 512],
                                     start=False, stop=False, perf_mode=DR)
                # bias pair for the -mean*rstd*colsum(wv) term (badd_v itself
                # folds into vb via sum(softmax)=1)
                nc.tensor.matmul(ps, xnT[:, KT:KT + 2, m * 128:(m + 1) * 128],
                                 wres_t[2][:, KT:KT + 2, n * 512:(n + 1) * 512],
                                 start=False, stop=True, perf_mode=DR)
                sl = slice(n * 512, (n + 1) * 512)
                nc.scalar.activation(out=vtmp[:, m, sl], in_=ps,
                                     func=AF.Identity, scale=1.0 / WS)
            nc.scalar.copy(out=v2_hi[:, m, :], in_=vtmp[:, m, :])

        def emit_Cq_stats(m):
            # emitted one m ahead of the rope half so the ACT sqrt never
            # blocks the next B evacuations queued behind it
            qm = qnat[:, m, :]
            sq = c_p.tile([128, GCOLS], BF16, tag="sqk")
            # DVE 2x makes the square cheap here; Pool already carries the
            # q-rope half and would over-subscribe
            nc.vector.tensor_mul(out=sq, in0=qm, in1=qm)
            ssq = c_small.tile([128, HG], BF16, tag="ssq")
            with nc.allow_low_precision(reason="bf16 rms stats; ~0.3% scale"):
                nc.vector.reduce_sum(out=ssq,
                                     in_=sq.rearrange("p (h d) -> p h d", h=HG),
                                     axis=mybir.AxisListType.X)
            rrq = c_small.tile([128, HG], BF16, tag="rrq")
            nc.scalar.activation(out=rrq, in_=ssq, func=AF.Abs_reciprocal_sqrt,
                                 bias=eps_t, scale=1.0 / HD)
            return rrq

        def emit_Cq_rope(m, rrq):
            qm = qnat[:, m, :]
            qmh = qm.rearrange("p (h c) -> p h c", h=HG)
            (ct, st) = cs_tiles[m]
            cb = ct.unsqueeze(1).broadcast_to([128, HG, RD // 2])
            sb_ = st.unsqueeze(1).broadcast_to([128, HG, RD // 2])
            if has_norm_w:
                nc.vector.tensor_mul(out=qmh, in0=qmh, in1=wqn_b)
            _emit_rope(qmh, cb, sb_, pool_half=True)
            rrq_b = rrq.unsqueeze(2).broadcast_to([128, HG, HD])
            nc.vector.tensor_mul(out=qmh, in0=qmh, in1=rrq_b)

        def emit_Ck_stats(m):
            km = knat[:, m, :]
            sk_ = c_p.tile([128, GCOLS], BF16, tag="sqk")
            # square on the idle gpsimd engine: DVE is the pacer here
            nc.gpsimd.tensor_mul(out=sk_, in0=km, in1=km)
            ssk = c_small.tile([128, HG], BF16, tag="ssk")
            with nc.allow_low_precision(reason="bf16 rms stats; ~0.3% scale"):
                nc.vector.reduce_sum(out=ssk,
                                     in_=sk_.rearrange("p (h d) -> p h d", h=HG),
                                     axis=mybir.AxisListType.X)
            nc.scalar.activation(out=rrk_all[:, m, :], in_=ssk,
                                 func=AF.Abs_reciprocal_sqrt, bias=eps128_t,
                                 scale=1.0)

        def emit_Ck_rope(m):
            km = knat[:, m, :]
            kmh = km.rearrange("p (h c) -> p h c", h=HG)
            (ct, st) = cs_tiles[m]
            cb = ct.unsqueeze(1).broadcast_to([128, HG, RD // 2])
            sb_ = st.unsqueeze(1).broadcast_to([128, HG, RD // 2])
            if has_norm_w:
                nc.vector.tensor_mul(out=kmh, in0=kmh, in1=wkn_b)
            _emit_rope(kmh, cb, sb_, pool_half=False)

        def _emit_rope(mh, cb, sb_, pool_half):
            # host-de-interleaved: cols [0:32]=x0, [32:64]=x1 per head.
            # pool_half routes the y1 branch to gpsimd (free in the q region).
            x0 = mh[:, :, 0:RD // 2]
            x1 = mh[:, :, RD // 2:RD]
            t0 = c_small.tile([128, HG, RD // 2], BF16, tag="t0")
            t1 = c_small.tile([128, HG, RD // 2], BF16, tag="t1")
            t2 = c_small.tile([128, HG, RD // 2], BF16, tag="t2")
            t3 = c_small.tile([128, HG, RD // 2], BF16, tag="t3")
            e = nc.gpsimd if pool_half else nc.vector
            nc.vector.tensor_mul(out=t0, in0=x0, in1=cb)
            nc.vector.tensor_mul(out=t1, in0=x1, in1=sb_)
            e.tensor_mul(out=t2, in0=x0, in1=sb_)
            e.tensor_mul(out=t3, in0=x1, in1=cb)
            nc.vector.tensor_sub(out=x0, in0=t0, in1=t1)
            e.tensor_add(out=x1, in0=t2, in1=t3)

        # ---- fused A + B-q emission ----
        xt01 = {i: emit_A_dma(i) for i in (0, 1)}
        init_misc()
        pend = {i: emit_A_stats(i, xt01.pop(i)) for i in (0, 1)}
        for i in range(NT):
            if i + 2 < NT:
                pend[i + 2] = emit_A_stats(i + 2)
            emit_A_body(i, *pend.pop(i))
            # all wq chunks MUST be emitted before the first emit_Bm below
            # (deps only order reads after already-emitted writes)
            if i == 0:
                load_wres(0, range(0, 2))
            elif i == 1:
                load_wres(0, range(2, 4))
            elif i == 3:
                load_wres(1)          # wk, consumed after A+B-q
            if i >= 1:
                emit_Bm_qk(0, qnat, i - 1, ("act", "act"))
        emit_Bm_qk(0, qnat, NT - 1, ("act", "act"))

        # deferred misc loads (consumed in phases C/E/F) — single batched DMAs
        # so their dispatch overhead doesn't contend with the x/w streams
        nc.sync.dma_start(out=gate_sb, in_=gate)
        nc.sync.dma_start(out=vb_sb, in_=vb)
        if has_norm_w:
            def _bcast_from_dram(ap, parts, reps=None):
                newap = [[0, parts]]
                if reps is not None:
                    newap.append([0, reps])
                newap += list(ap.ap)
                return bass.AP(tensor=ap.tensor, offset=ap.offset, ap=newap)
            nc.sync.dma_start(out=wqn_b, in_=_bcast_from_dram(wqn, 128, reps=HG))
            nc.sync.dma_start(out=wkn_b, in_=_bcast_from_dram(wkn, 128, reps=HG))
        nc.sync.dma_start(out=cos_all, in_=cos.rearrange("(m p) f -> p m f", p=128))
        nc.sync.dma_start(out=sin_all, in_=sin.rearrange("(m p) f -> p m f", p=128))

        # ---- B-k (PE) interleaved with C-q (DVE/Pool) per m-tile ----
        rrq_pend = {0: emit_Cq_stats(0)}
        for m in range(NT):
            emit_Bm_qk(1, knat, m, ("act", "act"))
            if m + 1 < NT:
                rrq_pend[m + 1] = emit_Cq_stats(m + 1)
            emit_Cq_rope(m, rrq_pend.pop(m))
            if m == 0:
                load_wres(2)          # wv_hi
                load_wres(3)          # wv_lo
        # ---- B-v (PE) interleaved with C-k (DVE/Pool) ----
        emit_Ck_stats(0)
        for m in range(NT):
            emit_Bm_v(m)
            if m + 1 < NT:
                emit_Ck_stats(m + 1)
            emit_Ck_rope(m)

        psb_cm.__exit__(None, None, None)
        pst_cm.__exit__(None, None, None)
        c_small_cm.__exit__(None, None, None)
        c_cm.__exit__(None, None, None)
        a_small_cm.__exit__(None, None, None)
        a_cm.__exit__(None, None, None)
        xnT_cm.__exit__(None, None, None)
        wres_cm.__exit__(None, None, None)

        # ======== phases D/E/F share the left stack ================================
        oT_cm = tc.tile_pool(name="oT", bufs=1, side="left")
        oT_p = oT_cm.__enter__()
        oT_hi = oT_p.tile([128, HG, S], F8)
        oT_lo = oT_p.tile([128, HG, S], F8)

        # wo prefetch (resident; consumed in phase F)
        wo_cm = tc.tile_pool(name="wo_res", bufs=1, side="left")
        wo_p = wo_cm.__enter__()
        wo_hi_sb = wo_p.tile([128, KT, HG, 128], F8)
        wo_lo_sb = wo_p.tile([128, KT, HG, 128], F8)

        # ---- phase D: transpose q, k -> [hd, s] per head (batched evacs)
        # q evacs on ACT (DVE is finishing C-k); k evacs on DVE; the v lo
        # split (SBUF-only, deferred from B-v) fills DVE alongside.
        qkT_cm = tc.tile_pool(name="qkT", bufs=1, side="left")
        qkT_p = qkT_cm.__enter__()
        qT = qkT_p.tile([128, HG, S], BF16)
        kT = qkT_p.tile([128, HG, S], BF16)
        pst2_cm = tc.tile_pool(name="ps_tr2", bufs=4, space="PSUM")
        pst2 = pst2_cm.__enter__()

        def emit_D(nat, dstT, h, evac):
            pt8 = pst2.tile([128, S], BF16, tag="pt8")
            for m in range(NT):
                nc.tensor.transpose(pt8[:, m * 128:(m + 1) * 128],
                                    nat[:, m, h * 128:(h + 1) * 128],
                                    ident_bf)
            dst = dstT[:, h, :]
            if evac == "act":
                nc.scalar.copy(out=dst, in_=pt8)
            else:
                nc.vector.tensor_copy(out=dst, in_=pt8)

        for h in range(HG):
            emit_D(qnat, qT, h, "act" if h % 2 == 0 else "dve")
            if h % 2 == 0:
                m = h // 2 * 2
                with nc.allow_low_precision(reason="fp8 v lo split"):
                    nc.vector.tensor_sub(out=v2_lo[:, m, :],
                                         in0=vtmp[:, m, :], in1=v2_hi[:, m, :])
                    nc.vector.tensor_sub(out=v2_lo[:, m + 1, :],
                                         in0=vtmp[:, m + 1, :],
                                         in1=v2_hi[:, m + 1, :])
        for h in range(HG):
            emit_D(knat, kT, h, "act" if h % 2 == 1 else "dve")
        pst2_cm.__exit__(None, None, None)
        # (natqk stays open through E/F: closing it between D and E would
        # add a pool barrier; SBUF has room)

        # wo loads (consumed in F; DMA-engine time hidden under phase E)
        for m in range(KT):
            nc.sync.dma_start(out=wo_hi_sb[:, m, :, :], in_=wo[:, m, 0, :, :])
            nc.sync.dma_start(out=wo_lo_sb[:, m, :, :], in_=wo[:, m, 1, :, :])

        # ---- phase E: attention per head (fp8 DoubleRow attn*v) ----
        at2_cm = tc.tile_pool(name="at2", bufs=2, side="left")
        at2_p = at2_cm.__enter__()
        rs_cm = tc.tile_pool(name="rsb", bufs=2, side="left")
        rs_p = rs_cm.__enter__()
        pssc_cm = tc.tile_pool(name="ps_sc", bufs=2, space="PSUM")
        pssc = pssc_cm.__enter__()
        pso_cm = tc.tile_pool(name="ps_o", bufs=1, space="PSUM")
        pso = pso_cm.__enter__()
        psdn_cm = tc.tile_pool(name="ps_dn", bufs=2, space="PSUM")
        psdn = psdn_cm.__enter__()

        def emit_scores(h, m):
            sc = pssc.tile([128, S], F32, tag="sc")
            lhs_k = kT[:, h, m * 128:(m + 1) * 128]
            nc.tensor.matmul(sc[:, 0:512], lhs_k, qT[:, h, 0:512],
                             start=True, stop=True)
            nc.tensor.matmul(sc[:, 512:1024], lhs_k, qT[:, h, 512:1024],
                             start=True, stop=True)
            return sc

        for h in range(HG):
            o_ps = pso.tile([128, S], F32, tag="o_ps", name=f"o_ps{h}")
            dns = [psdn.tile([128, 512], F32, tag="dn", name=f"dn{h}_{g}")
                   for g in range(2)]
            sc_next = emit_scores(h, 0)
            at = None
            for m in range(NT):
                sc = sc_next
                sc_next = emit_scores(h, m + 1) if m + 1 < NT else None
                if m % 2 == 0:
                    at = at2_p.tile([128, 2, S], F8, tag="at", name="at")
                # fp8 attn weights: exp(score*rrk - C) stays under e4m3 240;
                # the e^-C factor cancels between numerator and denominator
                nc.scalar.activation(out=at[:, m % 2, :], in_=sc, func=AF.Exp,
                                     scale=rrk_all[:, m, h:h + 1], bias=negc_t)
                if m % 2 == 1:
                    mp = m // 2
                    first, last = (mp == 0), (mp == NT // 2 - 1)
                    for g in range(2):
                        # denominator on PE: ones^T @ at, replicated rows
                        nc.tensor.matmul(dns[g], ones8,
                                         at[:, :, g * 512:(g + 1) * 512],
                                         start=first, stop=last, perf_mode=DR)
                    for vi, vsrc in enumerate((v2_hi, v2_lo)):
                        v_mh = vsrc[:, 2 * mp:2 * mp + 2, h * 128:(h + 1) * 128]
                        nc.tensor.matmul(o_ps[:, 0:512], v_mh, at[:, :, 0:512],
                                         start=(first and vi == 0),
                                         stop=(last and vi == 1), perf_mode=DR)
                        nc.tensor.matmul(o_ps[:, 512:1024], v_mh,
                                         at[:, :, 512:1024],
                                         start=(first and vi == 0),
                                         stop=(last and vi == 1), perf_mode=DR)
            for g in range(2):
                sl = slice(g * 512, (g + 1) * 512)
                sums = rs_p.tile([128, 512], F32, tag="sums", name=f"sums{h}{g}")
                nc.vector.reciprocal(out=sums, in_=dns[g])
                obf = rs_p.tile([128, 512], BF16, tag="obf", name=f"obf{h}{g}")
                nc.vector.tensor_mul(out=obf, in0=o_ps[:, sl], in1=sums)
                nc.vector.tensor_copy(out=oT_hi[:, h, sl], in_=obf)
                with nc.allow_low_precision(reason="fp8 o lo split"):
                    nc.vector.tensor_sub(out=oT_lo[:, h, sl], in0=obf,
                                         in1=oT_hi[:, h, sl])

        # ---- phase F: out projection as three fp8 DoubleRow chains
        # (hi*hi + hi*lo + lo*hi; the dropped lo*lo term is ~0.1%^2), emitted
        # inside E's psum pools so no pool barrier splits E/F
        F_CHAINS = 3
        F_KP = HG // 2

        def emit_F_mm(po, m, ci, kp):
            wsrc, osrc = ((wo_hi_sb, oT_hi), (wo_hi_sb, oT_lo),
                          (wo_lo_sb, oT_hi))[ci]
            first = (ci == 0 and kp == 0)
            last = (ci == F_CHAINS - 1 and kp == F_KP - 1)
            lhs = wsrc[:, m, 2 * kp:2 * kp + 2, :]
            nc.tensor.matmul(po[:, 0:512], lhs, osrc[:, 2 * kp:2 * kp + 2, 0:512],
                             start=first, stop=last, perf_mode=DR)
            nc.tensor.matmul(po[:, 512:1024], lhs,
                             osrc[:, 2 * kp:2 * kp + 2, 512:1024],
                             start=first, stop=last, perf_mode=DR)

        def emit_F_evac(po, m):
            ot_t = rs_p.tile([128, S], BF16, tag="ot_t", name=f"ot_t{m}")
            if m == KT - 1:
                # the kernel's tail: halve the evac+store chain so the first
                # half's DMA overlaps the second half's evacuation
                for g in (0, 1):
                    sl = slice(g * 512, (g + 1) * 512)
                    nc.vector.tensor_scalar(out=ot_t[:, sl], in0=po[:, sl],
                                            scalar1=gate_sb[:, m:m + 1],
                                            scalar2=vb_sb[:, m:m + 1],
                                            op0=ALU.mult, op1=ALU.add)
                    nc.sync.dma_start(out=out_t[m * 128:(m + 1) * 128, sl],
                                      in_=ot_t[:, sl])
                return
            if m % 2 == 0:
                nc.scalar.activation(out=ot_t, in_=po, func=AF.Identity,
                                     bias=vb_sb[:, m:m + 1],
                                     scale=gate_sb[:, m:m + 1])
            else:
                nc.vector.tensor_scalar(out=ot_t, in0=po,
                                        scalar1=gate_sb[:, m:m + 1],
                                        scalar2=vb_sb[:, m:m + 1],
                                        op0=ALU.mult, op1=ALU.add)
            nc.sync.dma_start(out=out_t[m * 128:(m + 1) * 128, :], in_=ot_t)

        def alloc_po(m):
            # alternate between the two dead attention psum pools for slack
            return (pssc.tile([128, S], F32, tag="sc", name=f"po{m}")
                    if m % 2 == 0
                    else pso.tile([128, S], F32, tag="o_ps", name=f"po{m}"))

        # first two output tiles interleave their chain loops so F has dense
        # matmuls queued before it needs the last head's normalization
        steps = [(ci, kp) for ci in range(F_CHAINS) for kp in range(F_KP)]
        po01 = {m: alloc_po(m) for m in (0, 1)}
        for m in (0, 1):
            for (ci, kp) in steps[:-2]:
                emit_F_mm(po01[m], m, ci, kp)
        for m in (0, 1):
            for (ci, kp) in steps[-2:]:
                emit_F_mm(po01[m], m, ci, kp)
            emit_F_evac(po01[m], m)
        for m in range(2, KT):
            po = alloc_po(m)
            for (ci, kp) in steps:
                emit_F_mm(po, m, ci, kp)
            emit_F_evac(po, m)

        psdn_cm.__exit__(None, None, None)
        pso_cm.__exit__(None, None, None)
        pssc_cm.__exit__(None, None, None)
        rs_cm.__exit__(None, None, None)
        at2_cm.__exit__(None, None, None)
        qkT_cm.__exit__(None, None, None)
        natqk_cm.__exit__(None, None, None)
        v_cm.__exit__(None, None, None)
        wo_cm.__exit__(None, None, None)
        oT_cm.__exit__(None, None, None)
        misc_cm.__exit__(None, None, None)

    nc.compile()
    return nc


_NC_CACHE = {}


def _get_nc(has_norm_w):
    if has_norm_w not in _NC_CACHE:
        _NC_CACHE[has_norm_w] = build_nc(has_norm_w)
    return _NC_CACHE[has_norm_w]


# per-head column permutation de-interleaving rope pairs:
# [0,2,..,62, 1,3,..,63, 64..127] within each head's 128 columns
def _qk_perm():
    base = np.concatenate([np.arange(0, RD, 2), np.arange(1, RD, 2),
                           np.arange(RD, HD)])
    return np.concatenate([h * HD + base for h in range(HG)])


_PERM = _qk_perm()
F8NP = ml_dtypes.float8_e4m3


def prep_in_maps(x, mod, cos, sin, qkv_w, qkv_b, mod_w, mod_b, out_w, out_b,
                 norm_q_w, norm_k_w):
    """Host-side sharding. Returns (in_maps, flags, x_np)."""
    x = np.asarray(x, dtype=np.float32)
    m3 = np.asarray(mod, np.float32) @ np.asarray(mod_w, np.float32) \
        + np.asarray(mod_b, np.float32)
    bias, scale, gatef = np.split(m3, 3, axis=-1)          # [B, D] each
    scale1p = (1.0 + scale).astype(np.float32)

    qkv_b = np.asarray(qkv_b, np.float32)
    has_norm_w = not (np.allclose(norm_q_w, 1.0) and np.allclose(norm_k_w, 1.0))

    cosb = np.ascontiguousarray(np.asarray(cos, np.float32).astype(ml_dtypes.bfloat16))
    sinb = np.ascontiguousarray(np.asarray(sin, np.float32).astype(ml_dtypes.bfloat16))
    qkv_w = np.asarray(qkv_w, np.float32)
    out_w = np.asarray(out_w, np.float32)
    out_b = np.asarray(out_b, np.float32)

    # per-head rope-pair de-interleave permutation (shared by q and k; inner
    # products and rms are invariant; cos/sin indices line up with x0/x1 halves)
    perm = _PERM
    hd_perm = np.concatenate([np.arange(0, RD, 2), np.arange(1, RD, 2),
                              np.arange(RD, HD)])

    def with_bias_tile(w_scaled_f8, badd, colsum_scaled):
        # tile16 rows (x ones): badd*WS/128; tile17 rows (x 16*nmr):
        # colsum(W_fp8)*WS / (128*16) — the rank-1 mean-subtraction term
        brow = np.broadcast_to((badd * (WS / 128.0))[None, :],
                               (128, badd.shape[0]))
        crow = np.broadcast_to((colsum_scaled / 2048.0)[None, :],
                               (128, badd.shape[0]))
        return np.ascontiguousarray(np.concatenate(
            [w_scaled_f8, brow.astype(F8NP), crow.astype(F8NP)], axis=0))

    in_maps = []
    for c in range(N_CORES):
        b, g = divmod(c, 2)
        lo = g * GCOLS
        s1p = scale1p[b][:, None]                          # [D, 1]
        wq_c = (qkv_w[:, lo:lo + GCOLS] * s1p)[:, perm]
        wk_c = (qkv_w[:, 2048 + lo:2048 + lo + GCOLS] * s1p)[:, perm]
        wv_c = qkv_w[:, 4096 + lo:4096 + lo + GCOLS] * s1p
        # modulation/qkv bias rows: b @ W (+ qkv bias), per projection
        badd_q = bias[b] @ qkv_w[:, lo:lo + GCOLS] + qkv_b[lo:lo + GCOLS]
        badd_k = bias[b] @ qkv_w[:, 2048 + lo:2048 + lo + GCOLS] \
            + qkv_b[2048 + lo:2048 + lo + GCOLS]
        badd_v = bias[b] @ qkv_w[:, 4096 + lo:4096 + lo + GCOLS] \
            + qkv_b[4096 + lo:4096 + lo + GCOLS]
        wo_c = out_w[lo:lo + GCOLS, :]                      # [GCOLS, D]
        # wo_l[p, m, kb, c] = wo_c[kb*128+p, m*128+c], fp8 hi/lo pair
        wo_l = np.transpose(
            wo_c.reshape(HG, 128, KT, 128), (1, 2, 0, 3)) * WS2
        wo_hi = wo_l.astype(F8NP)
        wo_lo = (wo_l - wo_hi.astype(np.float32)).astype(F8NP)
        wo_pair = np.stack([wo_hi, wo_lo], axis=2)   # [128, KT, 2, HG, 128]
        # v weights as an fp8 hi+lo pair at a shared scale: lo lands in the
        # e4m3 subnormal range, leaving ~0.1% net weight error
        wvh_c = (wv_c * WS).astype(F8NP)
        wvl_c = (wv_c * WS - wvh_c.astype(np.float32)).astype(F8NP)
        zrow = np.zeros((256, GCOLS), np.float32)
        wq_f8 = (wq_c * WS).astype(F8NP)
        wk_f8 = (wk_c * WS).astype(F8NP)
        # v bias folds into the output bias: sum(softmax)=1 makes o shift by
        # badd_v exactly, i.e. the final out shifts by (badd_v @ wo) * gate
        vb_full = badd_v @ wo_c                             # [D]
        if g == 0:
            vb_full = vb_full + out_b
        vbf = (vb_full * gatef[b]).astype(np.float32)
        im = {
            "x": np.ascontiguousarray(x[b].astype(ml_dtypes.bfloat16)),
            "cos": cosb, "sin": sinb,
            "wq": with_bias_tile(wq_f8, badd_q[perm],
                                 wq_f8.astype(np.float32).sum(0)),
            "wk": with_bias_tile(wk_f8, badd_k[perm],
                                 wk_f8.astype(np.float32).sum(0)),
            "wvh": with_bias_tile(wvh_c, np.zeros(GCOLS, np.float32),
                                  wvh_c.astype(np.float32).sum(0)
                                  + wvl_c.astype(np.float32).sum(0)),
            "wvl": np.ascontiguousarray(
                np.concatenate([wvl_c, zrow.astype(F8NP)], axis=0)),
            "wo": np.ascontiguousarray(wo_pair),
            "gate": np.ascontiguousarray(
                (gatef[b] / WS2).reshape(KT, 128).T),
            "vb": np.ascontiguousarray(vbf.reshape(KT, 128).T),
        }
        if has_norm_w:
            im["wqn"] = np.ascontiguousarray(
                np.asarray(norm_q_w, np.float32)[hd_perm].astype(ml_dtypes.bfloat16))
            im["wkn"] = np.ascontiguousarray(
                np.asarray(norm_k_w, np.float32)[hd_perm].astype(ml_dtypes.bfloat16))
        in_maps.append(im)
    return in_maps, has_norm_w, x


def gather(results, x):
    B = x.shape[0]
    outs = []
    for b in range(B):
        p = results[2 * b]["out_t"].astype(np.float32) \
            + results[2 * b + 1]["out_t"].astype(np.float32)        # [D, S]
        outs.append(p.T + x[b])
    return np.stack(outs).astype(np.float32)


def kernel(**inputs) -> np.ndarray:
    in_maps, has_norm_w, x = prep_in_maps(**inputs)
    nc = _get_nc(has_norm_w)
    res = run_bass_kernel_spmd(nc, in_maps, core_ids=list(range(N_CORES)))
    return gather(res.results, x)


if __name__ == "__main__":
    import time
    t0 = time.time()
    nc = build_nc(False)
    print("build+compile ok in", time.time() - t0, "s")
